# revision 1
# baseline (speedup 1.0000x reference)
"""CornerPool block (conv/BN/cummax-pool residual block) on 8 Trainium2
NeuronCores, pure data-parallel over batch (1 sample per core).

Reference computation per sample (x: [256, 128, 128] f32):
    res    = BN(conv1x1(x, w_res))
    p1     = relu(BN(conv3x3(x, w_vpre)))        # 256 -> 64
    pool1  = reverse-cummax(p1, axis=H)          # TopPool
    p2     = relu(BN(conv3x3(x, w_hpre)))        # 256 -> 64
    pool2  = reverse-cummax(p2, axis=W)          # LeftPool
    merged = BN(conv3x3(pool1 + pool2, w_add))   # 64 -> 256
    out    = relu(res + merged)
    y      = relu(BN(conv3x3(out, w_post)))      # 256 -> 256

Kernel strategy (per core):
  * BN folded into conv weights/biases host-side; every conv is a
    sum-of-9-shifted-taps matmul accumulation in PSUM (channels on the
    partition dim, pixels on the free dim, N=512 = 4 image rows).
  * vpre+hpre convs fused into one matmul stream (same rhs windows,
    64+64 output channels fill the 128-wide stationary operand).
  * Pooling as in-place DVE tensor_max scans on the padded [128,130,130]
    conv-output buffer (p1 on partitions 0:64, p2 on 64:128).
  * The merged conv contracts over all 128 partitions with the 64-row
    weight block replicated, which computes conv(pool1 + pool2) without
    materializing the sum.
  * res 1x1 conv re-reads the phase-A x strips still live in the SBUF
    ring; accumulates into the same PSUM group as the merged conv.
  * out is bounced through DRAM in 4-row strips; the post conv streams
    it back with halo. All phases are emitted interleaved in reverse
    strip order so the Tile scheduler overlaps them into one wavefront.
  * All matmuls use float32r (full fp32 data, 1 cycle/row at N=512).
"""

import sys

import numpy as np

if "/opt/trn_rl_repo" not in sys.path:
    sys.path.insert(0, "/opt/trn_rl_repo")

EPS = 1e-5
C, M = 256, 64
B, H, W = 8, 128, 128
S = 4                      # output rows per strip
NS = H // S                # 32 strips
HP, WP = H + 2, W + 2      # padded spatial dims
N_CORES = 8

_CACHE = {}


def _patch_tile_drain():
    """This walrus build rejects >2 packed sync waits on the TileContext
    exit Drain. Split them into standalone wait_ge instructions."""
    import concourse.tile as tile
    from concourse.vector_clock import ScopedClock

    if getattr(tile.TileContext._drain_and_barrier, "_split_waits", False):
        return

    def _drain_and_barrier(self, tick_clock, wait_clock):
        nc = self.nc
        probe = nc.sync.nop(nofuse=True)
        wait_clock.add_sem_waits(
            probe.ins, ScopedClock({None: tick_clock.global_clock})
        )
        waits = list(probe.ins.sync_info.on_wait)
        if len(waits) > 1:
            probe.ins.sync_info.on_wait = waits[:1]
            sems_by_id = {s.num: s for s in wait_clock.sems.allocated().values()}
            for w in waits[1:]:
                nc.sync.wait_ge(sems_by_id[w.id], w.wait_value)
        nc.sync.drain()
        nc.all_engine_barrier()
        popped = nc._tile_sem_poison_stack.pop()
        assert popped is self._sem_poison
        nc.clear_and_free_semaphores(list(self.sems.allocated().values()))
        nc.all_engine_barrier()

    _drain_and_barrier._split_waits = True
    tile.TileContext._drain_and_barrier = _drain_and_barrier


TAPS = [(dy, dx) for dy in range(3) for dx in range(3)]


def _legalize_waits(nc, mybir):
    """This walrus build accepts at most ONE sync wait per instruction
    (any class). Split excess waits into single-wait NoOps emitted just
    before the instruction on the same engine sequencer."""
    for f in nc.m.functions:
        for bb in f.blocks:
            insts = bb.instructions
            out = []
            for inst in insts:
                si = inst.sync_info
                waits = list(si.on_wait) if si is not None else []
                if len(waits) > 1:
                    for j, w in enumerate(waits[:-1]):
                        noop = mybir.InstNoOp(
                            name=f"{inst.name}-ws{j}",
                            sync_info=mybir.SyncInfo(on_wait=[w], on_update=[]),
                            bass_nofuse=True,
                            engine=inst.engine,
                        )
                        nc.register_instruction(noop)
                        out.append(noop)
                    si.on_wait = waits[-1:]
                out.append(inst)
            insts[:] = out


def build_nc(debug_taps=False):
    import concourse.bass as bass
    import concourse.mybir as mybir
    import concourse.tile as tile

    _patch_tile_drain()
    f32 = mybir.dt.float32
    f32r = mybir.dt.float32r
    Relu = mybir.ActivationFunctionType.Relu

    nc = bass.Bass()
    x_d = nc.declare_dram_parameter("x_s", [C, H, WP], f32r, isOutput=False)
    # lhsT weight banks, laid out [k(part), idx, m]
    wvh_d = nc.declare_dram_parameter("w_vh", [128, 18, 128], f32r, isOutput=False)
    wres_d = nc.declare_dram_parameter("w_res_l", [128, 4, 128], f32r, isOutput=False)
    wmrg_d = nc.declare_dram_parameter("w_mrg", [128, 18, 128], f32r, isOutput=False)
    wpost_d = nc.declare_dram_parameter("w_post_l", [128, 36, 128], f32r, isOutput=False)
    bias_d = nc.declare_dram_parameter("biases", [128, 5], f32, isOutput=False)
    zeros_d = nc.declare_dram_parameter("zeros", [128, 4 * WP], f32r, isOutput=False)
    y_d = nc.declare_dram_parameter("y", [C, H, W], f32, isOutput=True)
    if debug_taps:
        dbg_pooled_d = nc.declare_dram_parameter(
            "dbg_pooled", [128, HP, WP], f32, isOutput=True)
        dbg_out_d = nc.declare_dram_parameter(
            "dbg_out", [2, 128, H, WP], f32, isOutput=True)

    with tile.TileContext(nc) as tc:
        with (
            tc.tile_pool(name="const", bufs=1) as constp,
            tc.tile_pool(name="big", bufs=1) as bigp,
            tc.tile_pool(name="stage", bufs=6) as stagep,
            tc.tile_pool(name="psum", bufs=8, space="PSUM") as psump,
            tc.tile_pool(name="dram", bufs=1, space="DRAM") as dramp,
        ):
            # DRAM bounce for `out` between the merge conv and the post
            # conv — a Tile-tracked DRAM tile so the strip DMAs get
            # read-after-write dependencies.
            outbuf_d = dramp.tile([2, 128, H, WP], f32r)
            # ---- constants on the phase-A critical path ----
            # Constants travel on the gpsimd SWDGE queues so they never
            # contend with the strip traffic on the 16 HWDGE queues.
            wvh = constp.tile([128, 18, 128], f32r)
            for j in range(0, 18, 3):
                nc.gpsimd.dma_start(wvh[:, j : j + 3, :], wvh_d[:, j : j + 3, :])
            bias = constp.tile([128, 5], f32)
            nc.gpsimd.dma_start(bias[:], bias_d[:])
            wres = constp.tile([128, 4, 128], f32r)
            wmrg = constp.tile([128, 18, 128], f32r)
            wpost = constp.tile([128, 36, 128], f32r)

            # ---- persistent buffers ----
            # conv-A output, padded; p1 on partitions 0:64, p2 on 64:128
            pooled = bigp.tile([128, HP, WP], f32r)

            def emit_deferred_consts():
                nc.gpsimd.dma_start(wres[:], wres_d[:])
                for j in range(0, 18, 5):
                    e = min(j + 5, 18)
                    nc.gpsimd.dma_start(wmrg[:, j:e, :], wmrg_d[:, j:e, :])
                for j in range(0, 36, 5):
                    e = min(j + 5, 36)
                    nc.gpsimd.dma_start(wpost[:, j:e, :], wpost_d[:, j:e, :])
                # Memset is not ISA-legal for f32r on this toolchain;
                # zero the conv pad regions via DMA from a zeros param.
                nc.gpsimd.dma_start(pooled[:, 0, :], zeros_d[:, :WP])
                nc.gpsimd.dma_start(pooled[:, HP - 1, :], zeros_d[:, :WP])
                nc.sync.dma_start(pooled[:, 1 : HP - 1, 0:1], zeros_d[:, : HP - 2])
                nc.sync.dma_start(pooled[:, 1 : HP - 1, WP - 1 : WP], zeros_d[:, : HP - 2])

            # x strip ring for phase A: 4 slots x 2 channel-tiles
            xbuf = [
                [bigp.tile([128, S + 2, WP], f32r, name=f"xbuf{j}_{kt}")
                 for kt in range(2)]
                for j in range(4)
            ]
            # C output staging ring: padded width, pad cols zeroed once so
            # the bounce DMAs stay contiguous end-to-end
            obuf = [bigp.tile([128, S, WP], f32r, name=f"obuf{j}")
                    for j in range(6)]
            for j in range(6):
                nc.sync.dma_start(obuf[j][:], zeros_d[:])

            # x strip ring for the res conv in phase C (full padded width
            # so the DMA stays contiguous; the matmul reads cols 1..128)
            cbuf = [
                [bigp.tile([128, S, WP], f32r, name=f"cbuf{j}_{kt}")
                 for kt in range(2)]
                for j in range(3)
            ]
            # out strip ring for the post conv: 3 slots x 2 channel-tiles
            dbuf = [
                [bigp.tile([128, S + 2, WP], f32r, name=f"dbuf{j}_{ct}")
                 for ct in range(2)]
                for j in range(3)
            ]

            def emit_A(s):
                """conv(x, [w_vpre|w_hpre]) + BN + relu for rows 4s..4s+3."""
                r = S * s
                xb = xbuf[s % 4]
                lo = max(0, r - 1)
                hi = min(H, r + S + 1)
                dst_lo = lo - (r - 1)
                for kt in range(2):
                    if s == 0:
                        # slot previously held a later strip's rows; row -1 pad
                        nc.sync.dma_start(xb[kt][:, 0, :], zeros_d[:, :WP])
                    elif s == NS - 1:
                        # first use of the slot: bottom halo row is pad
                        nc.sync.dma_start(xb[kt][:, S + 1, :], zeros_d[:, :WP])
                    nc.sync.dma_start(
                        xb[kt][:, dst_lo : dst_lo + (hi - lo), :],
                        x_d[kt * 128 : (kt + 1) * 128, lo:hi, :],
                    )
                ps = psump.tile([128, S * W], f32, tag="ps")
                n = len(TAPS) * 2
                i = 0
                for kt in range(2):
                    for t, (dy, dx) in enumerate(TAPS):
                        nc.tensor.matmul(
                            ps[:],
                            wvh[:, kt * 9 + t, :],
                            xb[kt][:, dy : dy + S, dx : dx + W],
                            start=(i == 0),
                            stop=(i == n - 1),
                        )
                        i += 1
                nc.scalar.activation(
                    pooled[:, r + 1 : r + 1 + S, 1 : 1 + W],
                    ps[:],
                    Relu,
                    bias=bias[:, 0:1],
                )

            def emit_toppool(s):
                r = S * s
                for y in range(min(H - 2, r + S - 1), r - 1, -1):
                    nc.vector.tensor_max(
                        pooled[0:64, y + 1, 1 : 1 + W],
                        pooled[0:64, y + 1, 1 : 1 + W],
                        pooled[0:64, y + 2, 1 : 1 + W],
                    )

            def emit_leftpool(s):
                # rows 4s .. 4s+31 (strips s..s+7 just completed)
                rlo, rhi = S * s + 1, S * s + 33
                for x in range(W - 2, -1, -1):
                    nc.vector.tensor_max(
                        pooled[64:128, rlo:rhi, x + 1],
                        pooled[64:128, rlo:rhi, x + 1],
                        pooled[64:128, rlo:rhi, x + 2],
                    )

            def emit_C(s):
                """res conv + merged conv + add + relu -> out_bounce strip."""
                r = S * s
                cb = cbuf[s % 3]
                for kt in range(2):
                    nc.sync.dma_start(
                        cb[kt][:],
                        x_d[kt * 128 : (kt + 1) * 128, r : r + S, :],
                    )
                for ct in range(2):
                    ps = psump.tile([128, S * W], f32, tag="ps")
                    for kt in range(2):
                        nc.tensor.matmul(
                            ps[:],
                            wres[:, ct * 2 + kt, :],
                            cb[kt][:, :, 1 : 1 + W],
                            start=(kt == 0),
                            stop=False,
                        )
                    for t, (dy, dx) in enumerate(TAPS):
                        nc.tensor.matmul(
                            ps[:],
                            wmrg[:, ct * 9 + t, :],
                            pooled[:, r + dy : r + dy + S, dx : dx + W],
                            start=False,
                            stop=(t == 8),
                        )
                    st = obuf[(2 * s + ct) % 6]
                    nc.scalar.activation(
                        st[:, :, 1 : 1 + W], ps[:], Relu,
                        bias=bias[:, 1 + ct : 2 + ct])
                    nc.sync.dma_start(outbuf_d[ct, :, r : r + S, :], st[:])

            def emit_D(s):
                """post conv + BN + relu -> y strip."""
                r = S * s
                db = dbuf[s % 3]
                lo = max(0, r - 1)
                hi = min(H, r + S + 1)
                dst_lo = lo - (r - 1)
                for ct in range(2):
                    if s == 0:
                        nc.sync.dma_start(db[ct][:, 0, :], zeros_d[:, :WP])
                    elif s == NS - 1:
                        nc.sync.dma_start(db[ct][:, S + 1, :], zeros_d[:, :WP])
                    nc.sync.dma_start(
                        db[ct][:, dst_lo : dst_lo + (hi - lo), :],
                        outbuf_d[ct, :, lo:hi, :],
                    )
                for co in range(2):
                    ps = psump.tile([128, S * W], f32, tag="ps")
                    i = 0
                    for kt in range(2):
                        for t, (dy, dx) in enumerate(TAPS):
                            nc.tensor.matmul(
                                ps[:],
                                wpost[:, co * 18 + kt * 9 + t, :],
                                db[kt][:, dy : dy + S, dx : dx + W],
                                start=(i == 0),
                                stop=(i == 17),
                            )
                            i += 1
                    st = stagep.tile([128, S * W], f32, tag="std")
                    nc.scalar.activation(st[:], ps[:], Relu, bias=bias[:, 3 + co : 4 + co])
                    nc.sync.dma_start(y_d[co * 128 : (co + 1) * 128, r : r + S, :], st[:])

            # Software-pipelined wavefront in groups of 8 strips,
            # processed bottom-up so the reverse-cummax chains unlock
            # consumers as early as possible. The C/D batches for group k
            # are emitted AFTER group k-1's conv-A strips: the PE then has
            # a full group of conv-A matmuls to chew on while the DVE
            # runs the 32-row LeftPool chunk the C batch is waiting for.
            def emit_group_A(k):
                for s in range(8 * k + 7, 8 * k - 1, -1):
                    emit_A(s)
                    emit_toppool(s)
                emit_leftpool(8 * k)

            def emit_group_CD(k):
                for s in range(min(NS - 1, 8 * k + 8), 8 * k, -1):
                    emit_C(s)
                d_hi = NS - 1 if k == 3 else 8 * k + 9
                for s in range(d_hi, 8 * k + 1, -1):
                    emit_D(s)

            emit_deferred_consts()
            emit_group_A(3)
            for k in range(3, -1, -1):
                if k > 0:
                    emit_group_A(k - 1)
                emit_group_CD(k)
            emit_C(0)
            emit_D(1)
            emit_D(0)
            if debug_taps:
                nc.sync.dma_start(dbg_pooled_d[:], pooled[:])
                nc.sync.dma_start(dbg_out_d[:], outbuf_d[:])

    _legalize_waits(nc, mybir)
    return nc


def _fold_bn(w, bn):
    """BN(conv(x, w)) == conv(x, w * s[co]) + t[co]."""
    g, b, m, v = bn[0], bn[1], bn[2], bn[3]
    s = g / np.sqrt(v + EPS)
    t = b - m * s
    return w * s[:, None, None, None], t


def _prep_inputs(x, w_res, bn_res, w_vpre, bn_vpre, w_hpre, bn_hpre,
                 w_add, bn_add, w_post, bn_post):
    x = np.asarray(x, np.float32)
    xp = np.zeros((B, C, H, WP), np.float32)
    xp[:, :, :, 1 : 1 + W] = x
    x = xp
    w_res_s, t_res = _fold_bn(np.asarray(w_res, np.float32), np.asarray(bn_res, np.float32))
    w_vpre_s, t_vpre = _fold_bn(np.asarray(w_vpre, np.float32), np.asarray(bn_vpre, np.float32))
    w_hpre_s, t_hpre = _fold_bn(np.asarray(w_hpre, np.float32), np.asarray(bn_hpre, np.float32))
    w_add_s, t_add = _fold_bn(np.asarray(w_add, np.float32), np.asarray(bn_add, np.float32))
    w_post_s, t_post = _fold_bn(np.asarray(w_post, np.float32), np.asarray(bn_post, np.float32))

    # w_vh[k, kt*9+t, m]: m<64 vpre, m>=64 hpre; lhsT[k, m] = w[m, kt*128+k, dy, dx]
    w_vh = np.zeros((128, 18, 128), np.float32)
    for kt in range(2):
        for t, (dy, dx) in enumerate(TAPS):
            blk = kt * 128
            w_vh[:, kt * 9 + t, 0:64] = w_vpre_s[:, blk : blk + 128, dy, dx].T
            w_vh[:, kt * 9 + t, 64:128] = w_hpre_s[:, blk : blk + 128, dy, dx].T

    # w_res_l[k, ct*2+kt, m] = w_res_s[ct*128+m, kt*128+k]
    w_res_l = np.zeros((128, 4, 128), np.float32)
    for ct in range(2):
        for kt in range(2):
            w_res_l[:, ct * 2 + kt, :] = w_res_s[
                ct * 128 : (ct + 1) * 128, kt * 128 : (kt + 1) * 128, 0, 0
            ].T

    # w_mrg[k, ct*9+t, m] = w_add_s[ct*128+m, k%64, dy, dx]  (row-replicated)
    w_mrg = np.zeros((128, 18, 128), np.float32)
    for ct in range(2):
        for t, (dy, dx) in enumerate(TAPS):
            blkT = w_add_s[ct * 128 : (ct + 1) * 128, :, dy, dx].T  # [64, 128]
            w_mrg[0:64, ct * 9 + t, :] = blkT
            w_mrg[64:128, ct * 9 + t, :] = blkT

    # w_post_l[k, co*18+kt*9+t, m] = w_post_s[co*128+m, kt*128+k, dy, dx]
    w_post_l = np.zeros((128, 36, 128), np.float32)
    for co in range(2):
        for kt in range(2):
            for t, (dy, dx) in enumerate(TAPS):
                w_post_l[:, co * 18 + kt * 9 + t, :] = w_post_s[
                    co * 128 : (co + 1) * 128, kt * 128 : (kt + 1) * 128, dy, dx
                ].T

    biases = np.zeros((128, 5), np.float32)
    biases[0:64, 0] = t_vpre
    biases[64:128, 0] = t_hpre
    t_mrg = t_res + t_add
    biases[:, 1] = t_mrg[0:128]
    biases[:, 2] = t_mrg[128:256]
    biases[:, 3] = t_post[0:128]
    biases[:, 4] = t_post[128:256]

    shared = {
        "zeros": np.zeros((128, 4 * WP), np.float32),
        "w_vh": w_vh,
        "w_res_l": w_res_l,
        "w_mrg": w_mrg,
        "w_post_l": w_post_l,
        "biases": biases,
    }
    return x, shared


def kernel(x, w_res, bn_res, w_vpre, bn_vpre, w_hpre, bn_hpre,
           w_add, bn_add, w_post, bn_post):
    from concourse.bass_utils import run_bass_kernel_spmd

    x, shared = _prep_inputs(x, w_res, bn_res, w_vpre, bn_vpre, w_hpre,
                             bn_hpre, w_add, bn_add, w_post, bn_post)

    if "nc" not in _CACHE:
        _CACHE["nc"] = build_nc()
    nc = _CACHE["nc"]

    in_maps = [dict(shared, x_s=np.ascontiguousarray(x[i])) for i in range(N_CORES)]
    res = run_bass_kernel_spmd(nc, in_maps, list(range(N_CORES)))
    return np.stack([res.results[i]["y"] for i in range(N_CORES)]).astype(np.float32)



# revision 3
# speedup vs baseline: 1.0916x; 1.0916x over previous
"""CornerPool block (conv/BN/cummax-pool residual block) on 8 Trainium2
NeuronCores, pure data-parallel over batch (1 sample per core).

Reference computation per sample (x: [256, 128, 128] f32):
    res    = BN(conv1x1(x, w_res))
    p1     = relu(BN(conv3x3(x, w_vpre)))        # 256 -> 64
    pool1  = reverse-cummax(p1, axis=H)          # TopPool
    p2     = relu(BN(conv3x3(x, w_hpre)))        # 256 -> 64
    pool2  = reverse-cummax(p2, axis=W)          # LeftPool
    merged = BN(conv3x3(pool1 + pool2, w_add))   # 64 -> 256
    out    = relu(res + merged)
    y      = relu(BN(conv3x3(out, w_post)))      # 256 -> 256

Kernel strategy (per core):
  * BN folded into conv weights/biases host-side; every conv is a
    sum-of-9-shifted-taps matmul accumulation in PSUM (channels on the
    partition dim, pixels on the free dim, N=512 = 4 image rows).
  * vpre+hpre convs fused into one matmul stream (same rhs windows,
    64+64 output channels fill the 128-wide stationary operand).
  * Pooling as in-place DVE tensor_max scans on the padded [128,130,130]
    conv-output buffer (p1 on partitions 0:64, p2 on 64:128).
  * The merged conv contracts over all 128 partitions with the 64-row
    weight block replicated, which computes conv(pool1 + pool2) without
    materializing the sum.
  * res 1x1 conv re-reads the phase-A x strips still live in the SBUF
    ring; accumulates into the same PSUM group as the merged conv.
  * out is bounced through DRAM in 4-row strips; the post conv streams
    it back with halo. All phases are emitted interleaved in reverse
    strip order so the Tile scheduler overlaps them into one wavefront.
  * All matmuls use float32r (full fp32 data, 1 cycle/row at N=512).
"""

import sys

import numpy as np

if "/opt/trn_rl_repo" not in sys.path:
    sys.path.insert(0, "/opt/trn_rl_repo")

EPS = 1e-5
C, M = 256, 64
B, H, W = 8, 128, 128
S = 4                      # output rows per strip
NS = H // S                # 32 strips
HP, WP = H + 2, W + 2      # padded spatial dims
N_CORES = 8

_CACHE = {}


def _patch_tile_drain():
    """This walrus build rejects >2 packed sync waits on the TileContext
    exit Drain. Split them into standalone wait_ge instructions."""
    import concourse.tile as tile
    from concourse.vector_clock import ScopedClock

    if getattr(tile.TileContext._drain_and_barrier, "_split_waits", False):
        return

    def _drain_and_barrier(self, tick_clock, wait_clock):
        nc = self.nc
        probe = nc.sync.nop(nofuse=True)
        wait_clock.add_sem_waits(
            probe.ins, ScopedClock({None: tick_clock.global_clock})
        )
        waits = list(probe.ins.sync_info.on_wait)
        if len(waits) > 1:
            probe.ins.sync_info.on_wait = waits[:1]
            sems_by_id = {s.num: s for s in wait_clock.sems.allocated().values()}
            for w in waits[1:]:
                nc.sync.wait_ge(sems_by_id[w.id], w.wait_value)
        nc.sync.drain()
        nc.all_engine_barrier()
        popped = nc._tile_sem_poison_stack.pop()
        assert popped is self._sem_poison
        nc.clear_and_free_semaphores(list(self.sems.allocated().values()))
        nc.all_engine_barrier()

    _drain_and_barrier._split_waits = True
    tile.TileContext._drain_and_barrier = _drain_and_barrier


TAPS = [(dy, dx) for dy in range(3) for dx in range(3)]


def _legalize_waits(nc, mybir):
    """This walrus build accepts at most ONE sync wait per instruction
    (any class). Split excess waits into single-wait NoOps emitted just
    before the instruction on the same engine sequencer."""
    for f in nc.m.functions:
        for bb in f.blocks:
            insts = bb.instructions
            out = []
            for inst in insts:
                si = inst.sync_info
                waits = list(si.on_wait) if si is not None else []
                if len(waits) > 1:
                    for j, w in enumerate(waits[:-1]):
                        noop = mybir.InstNoOp(
                            name=f"{inst.name}-ws{j}",
                            sync_info=mybir.SyncInfo(on_wait=[w], on_update=[]),
                            bass_nofuse=True,
                            engine=inst.engine,
                        )
                        nc.register_instruction(noop)
                        out.append(noop)
                    si.on_wait = waits[-1:]
                out.append(inst)
            insts[:] = out


def build_nc(debug_taps=False):
    import concourse.bass as bass
    import concourse.mybir as mybir
    import concourse.tile as tile

    _patch_tile_drain()
    f32 = mybir.dt.float32
    f32r = mybir.dt.float32r
    Relu = mybir.ActivationFunctionType.Relu

    nc = bass.Bass()
    x_d = nc.declare_dram_parameter("x_s", [C, H, WP], f32r, isOutput=False)
    # lhsT weight banks, laid out [k(part), idx, m]
    wvh_d = nc.declare_dram_parameter("w_vh", [128, 18, 128], f32r, isOutput=False)
    wres_d = nc.declare_dram_parameter("w_res_l", [128, 4, 128], f32r, isOutput=False)
    wmrg_d = nc.declare_dram_parameter("w_mrg", [128, 18, 128], f32r, isOutput=False)
    wpost_d = nc.declare_dram_parameter("w_post_l", [128, 36, 128], f32r, isOutput=False)
    bias_d = nc.declare_dram_parameter("biases", [128, 5], f32, isOutput=False)
    zeros_d = nc.declare_dram_parameter("zeros", [128, 4 * WP], f32r, isOutput=False)
    y_d = nc.declare_dram_parameter("y", [C, H, W], f32, isOutput=True)
    if debug_taps:
        dbg_pooled_d = nc.declare_dram_parameter(
            "dbg_pooled", [128, HP, WP], f32, isOutput=True)

    AluMax = mybir.AluOpType.max
    AluBypass = mybir.AluOpType.bypass
    NDB = 5                    # out-strip ring slots per channel-tile

    with tile.TileContext(nc) as tc:
        with (
            tc.tile_pool(name="const", bufs=1) as constp,
            tc.tile_pool(name="big", bufs=1) as bigp,
            tc.tile_pool(name="stage", bufs=6) as stagep,
            tc.tile_pool(name="psum", bufs=8, space="PSUM") as psump,
        ):
            # ---- constants on the phase-A critical path ----
            # Constants travel on the gpsimd SWDGE queues so they never
            # contend with the strip traffic on the 16 HWDGE queues.
            wvh = constp.tile([128, 18, 128], f32r)
            for j in range(0, 18, 3):
                nc.gpsimd.dma_start(wvh[:, j : j + 3, :], wvh_d[:, j : j + 3, :])
            bias = constp.tile([128, 5], f32)
            nc.gpsimd.dma_start(bias[:], bias_d[:])
            # SBUF zero strip: source for all pad-region fills (DVE copies;
            # memset is not ISA-legal for f32r, and strided zero-DMAs from
            # DRAM are catastrophically slow on the sync queue).
            zbuf = constp.tile([128, WP], f32r)
            nc.gpsimd.dma_start(zbuf[:], zeros_d[:, :WP])
            wres = constp.tile([128, 4, 128], f32r)
            wmrg = constp.tile([128, 18, 128], f32r)
            wpost = constp.tile([128, 36, 128], f32r)

            # ---- persistent buffers ----
            # conv-A output, padded; p1 on partitions 0:64, p2 on 64:128
            pooled = bigp.tile([128, HP, WP], f32r)

            def emit_deferred_consts():
                # conv pad rows/cols of `pooled`: rows via cheap contiguous
                # DMAs, the two pad columns via DVE copies from zbuf.
                nc.gpsimd.dma_start(pooled[:, 0, :], zeros_d[:, :WP])
                nc.gpsimd.dma_start(pooled[:, HP - 1, :], zeros_d[:, :WP])
                nc.vector.tensor_max(
                    pooled[:, 1 : HP - 1, 0], zbuf[:, : HP - 2], zbuf[:, : HP - 2])
                nc.vector.tensor_max(
                    pooled[:, 1 : HP - 1, WP - 1], zbuf[:, : HP - 2], zbuf[:, : HP - 2])
                nc.gpsimd.dma_start(wres[:], wres_d[:])
                for j in range(0, 18, 5):
                    e = min(j + 5, 18)
                    nc.gpsimd.dma_start(wmrg[:, j:e, :], wmrg_d[:, j:e, :])
                for j in range(0, 36, 5):
                    e = min(j + 5, 36)
                    nc.gpsimd.dma_start(wpost[:, j:e, :], wpost_d[:, j:e, :])

            # x strip ring for phase A: 4 slots x 2 channel-tiles
            xbuf = [
                [bigp.tile([128, S + 2, WP], f32r, name=f"xbuf{j}_{kt}")
                 for kt in range(2)]
                for j in range(4)
            ]
            # x strip ring for the res conv in phase C (full padded width
            # so the DMA stays contiguous; the matmul reads cols 1..128)
            cbuf = [
                [bigp.tile([128, S, WP], f32r, name=f"cbuf{j}_{kt}")
                 for kt in range(2)]
                for j in range(3)
            ]
            # out strip ring between the merge conv and the post conv:
            # SBUF-resident, haloed. Tile j%NDB for strip j holds out rows
            # 4j-1 .. 4j+4 (indices 0..5); written directly by the C-phase
            # activations (no DRAM bounce), read by the D-phase matmuls.
            dbuf = [
                [bigp.tile([128, S + 2, WP], f32r, name=f"dbuf{j}_{ct}")
                 for ct in range(2)]
                for j in range(NDB)
            ]
            # dbuf pad columns 0 / WP-1 are read by the D-phase taps and
            # never written by the activations: zero them once.
            for j in range(NDB):
                for ct in range(2):
                    nc.vector.tensor_max(
                        dbuf[j][ct][:, :, 0], zbuf[:, : S + 2], zbuf[:, : S + 2])
                    nc.vector.tensor_max(
                        dbuf[j][ct][:, :, WP - 1], zbuf[:, : S + 2], zbuf[:, : S + 2])

            def emit_A(s):
                """conv(x, [w_vpre|w_hpre]) + BN + relu for rows 4s..4s+3,
                then the per-row LeftPool scans on the p2 half."""
                r = S * s
                xb = xbuf[s % 4]
                lo = max(0, r - 1)
                hi = min(H, r + S + 1)
                dst_lo = lo - (r - 1)
                for kt in range(2):
                    if s == 0:
                        # slot previously held a later strip's rows; row -1 pad
                        nc.vector.tensor_max(xb[kt][:, 0, :], zbuf[:], zbuf[:])
                    elif s == NS - 1:
                        # first use of the slot: bottom halo row is pad
                        nc.vector.tensor_max(xb[kt][:, S + 1, :], zbuf[:], zbuf[:])
                    nc.sync.dma_start(
                        xb[kt][:, dst_lo : dst_lo + (hi - lo), :],
                        x_d[kt * 128 : (kt + 1) * 128, lo:hi, :],
                    )
                ps = psump.tile([128, S * W], f32, tag="ps")
                n = len(TAPS) * 2
                i = 0
                for kt in range(2):
                    for t, (dy, dx) in enumerate(TAPS):
                        nc.tensor.matmul(
                            ps[:],
                            wvh[:, kt * 9 + t, :],
                            xb[kt][:, dy : dy + S, dx : dx + W],
                            start=(i == 0),
                            stop=(i == n - 1),
                        )
                        i += 1
                nc.scalar.activation(
                    pooled[:, r + 1 : r + 1 + S, 1 : 1 + W],
                    ps[:],
                    Relu,
                    bias=bias[:, 0:1],
                )
                # LeftPool (reverse cummax over W) for this strip's rows as
                # independent per-row hardware scans: no serial chain, and
                # p2 rows are final as soon as their strip lands.
                for ry in range(r + 1, r + 1 + S):
                    ap = pooled[64:128, ry, W : 0 : -1]
                    nc.vector.tensor_tensor_scan(
                        ap, ap, ap, 0.0, AluMax, AluBypass)

            def emit_toppool(s):
                r = S * s
                for y in range(min(H - 2, r + S - 1), r - 1, -1):
                    nc.vector.tensor_max(
                        pooled[0:64, y + 1, 1 : 1 + W],
                        pooled[0:64, y + 1, 1 : 1 + W],
                        pooled[0:64, y + 2, 1 : 1 + W],
                    )

            def emit_C(s):
                """res conv + merged conv + add + relu -> out ring tiles."""
                r = S * s
                cb = cbuf[s % 3]
                for kt in range(2):
                    nc.gpsimd.dma_start(
                        cb[kt][:],
                        x_d[kt * 128 : (kt + 1) * 128, r : r + S, :],
                    )
                for ct in range(2):
                    ps = psump.tile([128, S * W], f32, tag="ps")
                    for kt in range(2):
                        nc.tensor.matmul(
                            ps[:],
                            wres[:, ct * 2 + kt, :],
                            cb[kt][:, :, 1 : 1 + W],
                            start=(kt == 0),
                            stop=False,
                        )
                    for t, (dy, dx) in enumerate(TAPS):
                        nc.tensor.matmul(
                            ps[:],
                            wmrg[:, ct * 9 + t, :],
                            pooled[:, r + dy : r + dy + S, dx : dx + W],
                            start=False,
                            stop=(t == 8),
                        )
                    b = bias[:, 1 + ct : 2 + ct]
                    # out rows 4s..4s+3 land in ring tile s (indices 1..4);
                    # row 4s also serves as tile s-1's bottom halo (index 5)
                    # and row 4s+3 as tile s+1's top halo (index 0).
                    nc.scalar.activation(
                        dbuf[s % NDB][ct][:, 1 : 1 + S, 1 : 1 + W],
                        ps[:], Relu, bias=b)
                    if s > 0:
                        nc.scalar.activation(
                            dbuf[(s - 1) % NDB][ct][:, S + 1, 1 : 1 + W],
                            ps[:, 0:W], Relu, bias=b)
                    if s < NS - 1:
                        nc.scalar.activation(
                            dbuf[(s + 1) % NDB][ct][:, 0, 1 : 1 + W],
                            ps[:, (S - 1) * W : S * W], Relu, bias=b)

            def emit_D(s):
                """post conv + BN + relu -> y strip."""
                r = S * s
                db = dbuf[s % NDB]
                for co in range(2):
                    ps = psump.tile([128, S * W], f32, tag="ps")
                    i = 0
                    for kt in range(2):
                        for t, (dy, dx) in enumerate(TAPS):
                            nc.tensor.matmul(
                                ps[:],
                                wpost[:, co * 18 + kt * 9 + t, :],
                                db[kt][:, dy : dy + S, dx : dx + W],
                                start=(i == 0),
                                stop=(i == 17),
                            )
                            i += 1
                    st = stagep.tile([128, S * W], f32, tag="std")
                    nc.scalar.activation(st[:], ps[:], Relu, bias=bias[:, 3 + co : 4 + co])
                    nc.scalar.dma_start(y_d[co * 128 : (co + 1) * 128, r : r + S, :], st[:])

            # Software-pipelined wavefront in groups of 8 strips, processed
            # bottom-up so the TopPool chain unlocks consumers as early as
            # possible. The C/D pairs for group k are emitted AFTER group
            # k-1's conv-A strips so the PE always has conv-A matmuls to
            # chew on while pools/activations for the C batch settle.
            def emit_group_A(k):
                for s in range(8 * k + 7, 8 * k - 1, -1):
                    emit_A(s)
                    emit_toppool(s)

            def emit_group_CD(k):
                top = min(NS - 1, 8 * k + 8)
                for s in range(top, 8 * k, -1):
                    emit_C(s)
                    if s + 1 <= NS - 1:
                        emit_D(s + 1)

            emit_deferred_consts()
            # pad rows of the out ring: tile NS-1 index 5 is out row H
            # (zero), tile 0 index 0 is out row -1 (zero).
            for ct in range(2):
                nc.vector.tensor_max(
                    dbuf[(NS - 1) % NDB][ct][:, S + 1, :], zbuf[:], zbuf[:])
            emit_group_A(3)
            for k in range(3, -1, -1):
                if k > 0:
                    emit_group_A(k - 1)
                emit_group_CD(k)
            emit_C(0)
            emit_D(1)
            for ct in range(2):
                nc.vector.tensor_max(dbuf[0][ct][:, 0, :], zbuf[:], zbuf[:])
            emit_D(0)
            if debug_taps:
                nc.sync.dma_start(dbg_pooled_d[:], pooled[:])

    _legalize_waits(nc, mybir)
    return nc


def _fold_bn(w, bn):
    """BN(conv(x, w)) == conv(x, w * s[co]) + t[co]."""
    g, b, m, v = bn[0], bn[1], bn[2], bn[3]
    s = g / np.sqrt(v + EPS)
    t = b - m * s
    return w * s[:, None, None, None], t


def _prep_inputs(x, w_res, bn_res, w_vpre, bn_vpre, w_hpre, bn_hpre,
                 w_add, bn_add, w_post, bn_post):
    x = np.asarray(x, np.float32)
    xp = np.zeros((B, C, H, WP), np.float32)
    xp[:, :, :, 1 : 1 + W] = x
    x = xp
    w_res_s, t_res = _fold_bn(np.asarray(w_res, np.float32), np.asarray(bn_res, np.float32))
    w_vpre_s, t_vpre = _fold_bn(np.asarray(w_vpre, np.float32), np.asarray(bn_vpre, np.float32))
    w_hpre_s, t_hpre = _fold_bn(np.asarray(w_hpre, np.float32), np.asarray(bn_hpre, np.float32))
    w_add_s, t_add = _fold_bn(np.asarray(w_add, np.float32), np.asarray(bn_add, np.float32))
    w_post_s, t_post = _fold_bn(np.asarray(w_post, np.float32), np.asarray(bn_post, np.float32))

    # w_vh[k, kt*9+t, m]: m<64 vpre, m>=64 hpre; lhsT[k, m] = w[m, kt*128+k, dy, dx]
    w_vh = np.zeros((128, 18, 128), np.float32)
    for kt in range(2):
        for t, (dy, dx) in enumerate(TAPS):
            blk = kt * 128
            w_vh[:, kt * 9 + t, 0:64] = w_vpre_s[:, blk : blk + 128, dy, dx].T
            w_vh[:, kt * 9 + t, 64:128] = w_hpre_s[:, blk : blk + 128, dy, dx].T

    # w_res_l[k, ct*2+kt, m] = w_res_s[ct*128+m, kt*128+k]
    w_res_l = np.zeros((128, 4, 128), np.float32)
    for ct in range(2):
        for kt in range(2):
            w_res_l[:, ct * 2 + kt, :] = w_res_s[
                ct * 128 : (ct + 1) * 128, kt * 128 : (kt + 1) * 128, 0, 0
            ].T

    # w_mrg[k, ct*9+t, m] = w_add_s[ct*128+m, k%64, dy, dx]  (row-replicated)
    w_mrg = np.zeros((128, 18, 128), np.float32)
    for ct in range(2):
        for t, (dy, dx) in enumerate(TAPS):
            blkT = w_add_s[ct * 128 : (ct + 1) * 128, :, dy, dx].T  # [64, 128]
            w_mrg[0:64, ct * 9 + t, :] = blkT
            w_mrg[64:128, ct * 9 + t, :] = blkT

    # w_post_l[k, co*18+kt*9+t, m] = w_post_s[co*128+m, kt*128+k, dy, dx]
    w_post_l = np.zeros((128, 36, 128), np.float32)
    for co in range(2):
        for kt in range(2):
            for t, (dy, dx) in enumerate(TAPS):
                w_post_l[:, co * 18 + kt * 9 + t, :] = w_post_s[
                    co * 128 : (co + 1) * 128, kt * 128 : (kt + 1) * 128, dy, dx
                ].T

    biases = np.zeros((128, 5), np.float32)
    biases[0:64, 0] = t_vpre
    biases[64:128, 0] = t_hpre
    t_mrg = t_res + t_add
    biases[:, 1] = t_mrg[0:128]
    biases[:, 2] = t_mrg[128:256]
    biases[:, 3] = t_post[0:128]
    biases[:, 4] = t_post[128:256]

    shared = {
        "zeros": np.zeros((128, 4 * WP), np.float32),
        "w_vh": w_vh,
        "w_res_l": w_res_l,
        "w_mrg": w_mrg,
        "w_post_l": w_post_l,
        "biases": biases,
    }
    return x, shared


def kernel(x, w_res, bn_res, w_vpre, bn_vpre, w_hpre, bn_hpre,
           w_add, bn_add, w_post, bn_post):
    from concourse.bass_utils import run_bass_kernel_spmd

    x, shared = _prep_inputs(x, w_res, bn_res, w_vpre, bn_vpre, w_hpre,
                             bn_hpre, w_add, bn_add, w_post, bn_post)

    if "nc" not in _CACHE:
        _CACHE["nc"] = build_nc()
    nc = _CACHE["nc"]

    in_maps = [dict(shared, x_s=np.ascontiguousarray(x[i])) for i in range(N_CORES)]
    res = run_bass_kernel_spmd(nc, in_maps, list(range(N_CORES)))
    return np.stack([res.results[i]["y"] for i in range(N_CORES)]).astype(np.float32)



# revision 12
# speedup vs baseline: 1.1786x; 1.0797x over previous
"""CornerPool block (conv/BN/cummax-pool residual block) on 8 Trainium2
NeuronCores, pure data-parallel over batch (1 sample per core).

Reference computation per sample (x: [256, 128, 128] f32):
    res    = BN(conv1x1(x, w_res))
    p1     = relu(BN(conv3x3(x, w_vpre)))        # 256 -> 64
    pool1  = reverse-cummax(p1, axis=H)          # TopPool
    p2     = relu(BN(conv3x3(x, w_hpre)))        # 256 -> 64
    pool2  = reverse-cummax(p2, axis=W)          # LeftPool
    merged = BN(conv3x3(pool1 + pool2, w_add))   # 64 -> 256
    out    = relu(res + merged)
    y      = relu(BN(conv3x3(out, w_post)))      # 256 -> 256

Kernel strategy (per core):
  * BN folded into conv weights/biases host-side; every conv is a
    sum-of-9-shifted-taps matmul accumulation in PSUM (channels on the
    partition dim, pixels on the free dim, N=512 = 4 image rows).
  * vpre+hpre convs fused into one matmul stream (same rhs windows,
    64+64 output channels fill the 128-wide stationary operand).
  * Pooling as in-place DVE tensor_max scans on the padded [128,130,130]
    conv-output buffer (p1 on partitions 0:64, p2 on 64:128).
  * The merged conv contracts over all 128 partitions with the 64-row
    weight block replicated, which computes conv(pool1 + pool2) without
    materializing the sum.
  * res 1x1 conv re-reads the phase-A x strips still live in the SBUF
    ring; accumulates into the same PSUM group as the merged conv.
  * out is bounced through DRAM in 4-row strips; the post conv streams
    it back with halo. All phases are emitted interleaved in reverse
    strip order so the Tile scheduler overlaps them into one wavefront.
  * All matmuls use float32r (full fp32 data, 1 cycle/row at N=512).
"""

import sys

import numpy as np

if "/opt/trn_rl_repo" not in sys.path:
    sys.path.insert(0, "/opt/trn_rl_repo")

EPS = 1e-5
C, M = 256, 64
B, H, W = 8, 128, 128
S = 4                      # output rows per strip
NS = H // S                # 32 strips
HP, WP = H + 2, W + 2      # padded spatial dims
N_CORES = 8

_CACHE = {}


def _patch_tile_drain():
    """This walrus build rejects >2 packed sync waits on the TileContext
    exit Drain. Split them into standalone wait_ge instructions."""
    import concourse.tile as tile
    from concourse.vector_clock import ScopedClock

    if getattr(tile.TileContext._drain_and_barrier, "_split_waits", False):
        return

    def _drain_and_barrier(self, tick_clock, wait_clock):
        nc = self.nc
        probe = nc.sync.nop(nofuse=True)
        wait_clock.add_sem_waits(
            probe.ins, ScopedClock({None: tick_clock.global_clock})
        )
        waits = list(probe.ins.sync_info.on_wait)
        if len(waits) > 1:
            probe.ins.sync_info.on_wait = waits[:1]
            sems_by_id = {s.num: s for s in wait_clock.sems.allocated().values()}
            for w in waits[1:]:
                nc.sync.wait_ge(sems_by_id[w.id], w.wait_value)
        nc.sync.drain()
        nc.all_engine_barrier()
        popped = nc._tile_sem_poison_stack.pop()
        assert popped is self._sem_poison
        nc.clear_and_free_semaphores(list(self.sems.allocated().values()))
        nc.all_engine_barrier()

    _drain_and_barrier._split_waits = True
    tile.TileContext._drain_and_barrier = _drain_and_barrier


TAPS = [(dy, dx) for dy in range(3) for dx in range(3)]
# merge-conv windows over the pool-sum buffer: (dy, 0) windows carry taps
# (dy,0) on the aligned half and (dy,1) on the col-shifted half; (dy, 2)
# windows carry tap (dy,2) with a zeroed shifted half.
MRG_WINS = [(0, 0), (1, 0), (2, 0), (0, 2), (1, 2), (2, 2)]


def _legalize_waits(nc, mybir):
    """This walrus build accepts at most ONE sync wait per instruction
    (any class). Split excess waits into single-wait NoOps emitted just
    before the instruction on the same engine sequencer."""
    for f in nc.m.functions:
        for bb in f.blocks:
            insts = bb.instructions
            out = []
            for inst in insts:
                si = inst.sync_info
                waits = list(si.on_wait) if si is not None else []
                if len(waits) > 1:
                    for j, w in enumerate(waits[:-1]):
                        noop = mybir.InstNoOp(
                            name=f"{inst.name}-ws{j}",
                            sync_info=mybir.SyncInfo(on_wait=[w], on_update=[]),
                            bass_nofuse=True,
                            engine=inst.engine,
                        )
                        nc.register_instruction(noop)
                        out.append(noop)
                    si.on_wait = waits[-1:]
                out.append(inst)
            insts[:] = out


def build_nc(debug_taps=False):
    import concourse.bass as bass
    import concourse.mybir as mybir
    import concourse.tile as tile

    _patch_tile_drain()
    f32 = mybir.dt.float32
    f32r = mybir.dt.float32r
    Relu = mybir.ActivationFunctionType.Relu

    nc = bass.Bass()
    x_d = nc.declare_dram_parameter("x_s", [C, H, WP], f32r, isOutput=False)
    # lhsT weight banks, laid out [k(part), idx, m]
    wvh_d = nc.declare_dram_parameter("w_vh", [128, 18, 128], f32r, isOutput=False)
    wres_d = nc.declare_dram_parameter("w_res_l", [128, 4, 128], f32r, isOutput=False)
    wmrg_d = nc.declare_dram_parameter("w_mrg", [128, 12, 128], f32r, isOutput=False)
    wpost_d = nc.declare_dram_parameter("w_post_l", [128, 36, 128], f32r, isOutput=False)
    bias_d = nc.declare_dram_parameter("biases", [128, 5], f32, isOutput=False)
    zeros_d = nc.declare_dram_parameter("zeros", [128, 4 * WP], f32r, isOutput=False)
    y_d = nc.declare_dram_parameter("y", [C, H, W], f32, isOutput=True)
    if debug_taps:
        dbg_pooled_d = nc.declare_dram_parameter(
            "dbg_pooled", [128, HP, WP], f32, isOutput=True)

    AluMax = mybir.AluOpType.max
    AluBypass = mybir.AluOpType.bypass
    NDB = 5                    # out-strip ring slots per channel-tile

    with tile.TileContext(nc) as tc:
        with (
            tc.tile_pool(name="const", bufs=1) as constp,
            tc.tile_pool(name="big", bufs=1) as bigp,
            tc.tile_pool(name="stage", bufs=6) as stagep,
            tc.tile_pool(name="psum", bufs=8, space="PSUM") as psump,
        ):
            # ---- constants on the phase-A critical path ----
            # Constants travel on the gpsimd SWDGE queues so they never
            # contend with the strip traffic on the 16 HWDGE queues.
            wvh = constp.tile([128, 18, 128], f32r)
            for j in range(0, 18, 3):
                nc.gpsimd.dma_start(wvh[:, j : j + 3, :], wvh_d[:, j : j + 3, :])
            bias = constp.tile([128, 5], f32)
            nc.gpsimd.dma_start(bias[:], bias_d[:])
            # SBUF zero strip: source for all pad-region fills (DVE copies;
            # memset is not ISA-legal for f32r, and strided zero-DMAs from
            # DRAM are catastrophically slow on the sync queue).
            zbuf = constp.tile([128, WP], f32r)
            nc.gpsimd.dma_start(zbuf[:], zeros_d[:, :WP])
            wres = constp.tile([128, 4, 128], f32r)
            wmrg = constp.tile([128, 12, 128], f32r)
            wpost = constp.tile([128, 36, 128], f32r)

            # ---- persistent buffers ----
            # conv-A output, padded; p1 on partitions 0:64, p2 on 64:128
            pooled = bigp.tile([128, HP, WP], f32r)

            def emit_deferred_consts():
                # conv pad rows/cols of `pooled`: rows via cheap contiguous
                # DMAs, the two pad columns via DVE copies from zbuf.
                nc.gpsimd.dma_start(pooled[:, 0, :], zeros_d[:, :WP])
                nc.gpsimd.dma_start(pooled[:, HP - 1, :], zeros_d[:, :WP])
                nc.vector.tensor_max(
                    pooled[:, 1 : HP - 1, 0], zbuf[:, : HP - 2], zbuf[:, : HP - 2])
                nc.vector.tensor_max(
                    pooled[:, 1 : HP - 1, WP - 1], zbuf[:, : HP - 2], zbuf[:, : HP - 2])
                nc.gpsimd.dma_start(wres[:], wres_d[:])
                for j in range(0, 12, 4):
                    e = min(j + 4, 12)
                    nc.gpsimd.dma_start(wmrg[:, j:e, :], wmrg_d[:, j:e, :])
                for j in range(0, 36, 5):
                    e = min(j + 5, 36)
                    nc.gpsimd.dma_start(wpost[:, j:e, :], wpost_d[:, j:e, :])

            # x strip ring for phase A: 4 slots x 2 channel-tiles
            xbuf = [
                [bigp.tile([128, S + 2, WP], f32r, name=f"xbuf{j}_{kt}")
                 for kt in range(2)]
                for j in range(4)
            ]
            # x strip ring for the res conv in phase C (full padded width
            # so the DMA stays contiguous; the matmul reads cols 1..128)
            cbuf = [
                [bigp.tile([128, S, WP], f32r, name=f"cbuf{j}_{kt}")
                 for kt in range(2)]
                for j in range(3)
            ]
            # staging ring for the pool-sum: p2 rows bounced through a
            # partition-remap DMA so the DVE can add them into the p1 half
            sbuf_tmp = [bigp.tile([128, S, WP], f32r, name=f"sum{j}")
                        for j in range(3)]
            # out strip ring between the merge conv and the post conv:
            # SBUF-resident, haloed. Tile j%NDB for strip j holds out rows
            # 4j-1 .. 4j+4 (indices 0..5); written directly by the C-phase
            # activations (no DRAM bounce), read by the D-phase matmuls.
            dbuf = [
                [bigp.tile([128, S + 2, WP], f32r, name=f"dbuf{j}_{ct}")
                 for ct in range(2)]
                for j in range(NDB)
            ]
            # dbuf pad columns 0 / WP-1 are read by the D-phase taps and
            # never written by the activations: zero them once.
            for j in range(NDB):
                for ct in range(2):
                    nc.vector.tensor_max(
                        dbuf[j][ct][:, :, 0], zbuf[:, : S + 2], zbuf[:, : S + 2])
                    nc.vector.tensor_max(
                        dbuf[j][ct][:, :, WP - 1], zbuf[:, : S + 2], zbuf[:, : S + 2])

            def emit_A(s):
                """conv(x, [w_vpre|w_hpre]) + BN + relu for rows 4s..4s+3,
                then the per-row LeftPool scans on the p2 half."""
                r = S * s
                xb = xbuf[s % 4]
                lo = max(0, r - 1)
                hi = min(H, r + S + 1)
                dst_lo = lo - (r - 1)
                for kt in range(2):
                    if s == 0:
                        # slot previously held a later strip's rows; row -1 pad
                        nc.vector.tensor_max(xb[kt][:, 0, :], zbuf[:], zbuf[:])
                    elif s == NS - 1:
                        # first use of the slot: bottom halo row is pad
                        nc.vector.tensor_max(xb[kt][:, S + 1, :], zbuf[:], zbuf[:])
                    nc.sync.dma_start(
                        xb[kt][:, dst_lo : dst_lo + (hi - lo), :],
                        x_d[kt * 128 : (kt + 1) * 128, lo:hi, :],
                    )
                ps = psump.tile([128, S * W], f32, tag="ps")
                n = len(TAPS) * 2
                i = 0
                for kt in range(2):
                    for t, (dy, dx) in enumerate(TAPS):
                        nc.tensor.matmul(
                            ps[:],
                            wvh[:, kt * 9 + t, :],
                            xb[kt][:, dy : dy + S, dx : dx + W],
                            start=(i == 0),
                            stop=(i == n - 1),
                        )
                        i += 1
                nc.scalar.activation(
                    pooled[:, r + 1 : r + 1 + S, 1 : 1 + W],
                    ps[:],
                    Relu,
                    bias=bias[:, 0:1],
                )
                # LeftPool (reverse cummax over W) for this strip's rows as
                # independent per-row hardware scans: no serial chain, and
                # p2 rows are final as soon as their strip lands.
                for ry in range(r + 1, r + 1 + S):
                    ap = pooled[64:128, ry, W : 0 : -1]
                    nc.vector.tensor_tensor_scan(
                        ap, ap, ap, 0.0, AluMax, AluBypass)

            def emit_toppool(s):
                r = S * s
                for y in range(min(H - 2, r + S - 1), r - 1, -1):
                    nc.vector.tensor_max(
                        pooled[0:64, y + 1, 1 : 1 + W],
                        pooled[0:64, y + 1, 1 : 1 + W],
                        pooled[0:64, y + 2, 1 : 1 + W],
                    )

            def emit_sum(m):
                """Collapse pooled rows 4m+1..4m+4 into the explicit pool
                sum: p1 half <- p1 + p2 (via a partition-remap bounce), then
                p2 half <- sum shifted one column left. The merge conv then
                contracts [sum(tap dx) ; sum(tap dx+1)] in one matmul, so 9
                taps need only 6 windows per output-channel tile."""
                rlo = S * m + 1
                t = sbuf_tmp[m % 3]
                nc.gpsimd.dma_start(t[0:64, :, :], pooled[64:128, rlo : rlo + S, :])
                nc.vector.tensor_add(
                    pooled[0:64, rlo : rlo + S, :],
                    pooled[0:64, rlo : rlo + S, :],
                    t[0:64, :, :],
                )
                nc.gpsimd.dma_start(
                    pooled[64:128, rlo : rlo + S, 0 : WP - 1],
                    pooled[0:64, rlo : rlo + S, 1:WP],
                )

            def emit_C(s):
                """res conv + merged conv + add + relu -> out ring tiles."""
                r = S * s
                cb = cbuf[s % 3]
                for kt in range(2):
                    nc.gpsimd.dma_start(
                        cb[kt][:],
                        x_d[kt * 128 : (kt + 1) * 128, r : r + S, :],
                    )
                for ct in range(2):
                    ps = psump.tile([128, S * W], f32, tag="ps")
                    for kt in range(2):
                        nc.tensor.matmul(
                            ps[:],
                            wres[:, ct * 2 + kt, :],
                            cb[kt][:, :, 1 : 1 + W],
                            start=(kt == 0),
                            stop=False,
                        )
                    for t, (dy, dx) in enumerate(MRG_WINS):
                        nc.tensor.matmul(
                            ps[:],
                            wmrg[:, ct * 6 + t, :],
                            pooled[:, r + dy : r + dy + S, dx : dx + W],
                            start=False,
                            stop=(t == 5),
                        )
                    b = bias[:, 1 + ct : 2 + ct]
                    # out rows 4s..4s+3 land in ring tile s (indices 1..4);
                    # row 4s also serves as tile s-1's bottom halo (index 5)
                    # and row 4s+3 as tile s+1's top halo (index 0).
                    nc.scalar.activation(
                        dbuf[s % NDB][ct][:, 1 : 1 + S, 1 : 1 + W],
                        ps[:], Relu, bias=b)
                    if s > 0:
                        nc.scalar.activation(
                            dbuf[(s - 1) % NDB][ct][:, S + 1, 1 : 1 + W],
                            ps[:, 0:W], Relu, bias=b)
                    if s < NS - 1:
                        nc.scalar.activation(
                            dbuf[(s + 1) % NDB][ct][:, 0, 1 : 1 + W],
                            ps[:, (S - 1) * W : S * W], Relu, bias=b)

            def emit_D(s):
                """post conv + BN + relu -> y strip."""
                r = S * s
                db = dbuf[s % NDB]
                for co in range(2):
                    ps = psump.tile([128, S * W], f32, tag="ps")
                    i = 0
                    for kt in range(2):
                        for t, (dy, dx) in enumerate(TAPS):
                            nc.tensor.matmul(
                                ps[:],
                                wpost[:, co * 18 + kt * 9 + t, :],
                                db[kt][:, dy : dy + S, dx : dx + W],
                                start=(i == 0),
                                stop=(i == 17),
                            )
                            i += 1
                    st = stagep.tile([128, S * W], f32, tag="std")
                    nc.scalar.activation(st[:], ps[:], Relu, bias=bias[:, 3 + co : 4 + co])
                    nc.scalar.dma_start(y_d[co * 128 : (co + 1) * 128, r : r + S, :], st[:])

            # Software-pipelined wavefront in groups of 8 strips, processed
            # bottom-up so the TopPool chain unlocks consumers as early as
            # possible. The C/D pairs for group k are emitted AFTER group
            # k-1's conv-A strips so the PE always has conv-A matmuls to
            # chew on while pools/activations for the C batch settle.
            def emit_group_A(k):
                for s in range(8 * k + 7, 8 * k - 1, -1):
                    emit_A(s)
                    emit_toppool(s)
                    # sum chunk s+1 is ready once toppool(s) has consumed
                    # the last original p1 row it overwrites
                    if s + 1 <= NS - 1:
                        emit_sum(s + 1)
                if k == 0:
                    emit_sum(0)

            def emit_group_CD(k):
                top = min(NS - 1, 8 * k + 8)
                for s in range(top, 8 * k, -1):
                    emit_C(s)
                    if s + 1 <= NS - 1:
                        emit_D(s + 1)

            emit_deferred_consts()
            # pad rows of the out ring: tile NS-1 index 5 is out row H
            # (zero), tile 0 index 0 is out row -1 (zero).
            for ct in range(2):
                nc.vector.tensor_max(
                    dbuf[(NS - 1) % NDB][ct][:, S + 1, :], zbuf[:], zbuf[:])
            emit_group_A(3)
            for k in range(3, -1, -1):
                if k > 0:
                    emit_group_A(k - 1)
                emit_group_CD(k)
            emit_C(0)
            emit_D(1)
            for ct in range(2):
                nc.vector.tensor_max(dbuf[0][ct][:, 0, :], zbuf[:], zbuf[:])
            emit_D(0)
            if debug_taps:
                nc.sync.dma_start(dbg_pooled_d[:], pooled[:])

    _legalize_waits(nc, mybir)
    return nc


def _fold_bn(w, bn):
    """BN(conv(x, w)) == conv(x, w * s[co]) + t[co]."""
    g, b, m, v = bn[0], bn[1], bn[2], bn[3]
    s = g / np.sqrt(v + EPS)
    t = b - m * s
    return w * s[:, None, None, None], t


def _prep_inputs(x, w_res, bn_res, w_vpre, bn_vpre, w_hpre, bn_hpre,
                 w_add, bn_add, w_post, bn_post):
    x = np.asarray(x, np.float32)
    xp = np.zeros((B, C, H, WP), np.float32)
    xp[:, :, :, 1 : 1 + W] = x
    x = xp
    w_res_s, t_res = _fold_bn(np.asarray(w_res, np.float32), np.asarray(bn_res, np.float32))
    w_vpre_s, t_vpre = _fold_bn(np.asarray(w_vpre, np.float32), np.asarray(bn_vpre, np.float32))
    w_hpre_s, t_hpre = _fold_bn(np.asarray(w_hpre, np.float32), np.asarray(bn_hpre, np.float32))
    w_add_s, t_add = _fold_bn(np.asarray(w_add, np.float32), np.asarray(bn_add, np.float32))
    w_post_s, t_post = _fold_bn(np.asarray(w_post, np.float32), np.asarray(bn_post, np.float32))

    # w_vh[k, kt*9+t, m]: m<64 vpre, m>=64 hpre; lhsT[k, m] = w[m, kt*128+k, dy, dx]
    w_vh = np.zeros((128, 18, 128), np.float32)
    for kt in range(2):
        for t, (dy, dx) in enumerate(TAPS):
            blk = kt * 128
            w_vh[:, kt * 9 + t, 0:64] = w_vpre_s[:, blk : blk + 128, dy, dx].T
            w_vh[:, kt * 9 + t, 64:128] = w_hpre_s[:, blk : blk + 128, dy, dx].T

    # w_res_l[k, ct*2+kt, m] = w_res_s[ct*128+m, kt*128+k]
    w_res_l = np.zeros((128, 4, 128), np.float32)
    for ct in range(2):
        for kt in range(2):
            w_res_l[:, ct * 2 + kt, :] = w_res_s[
                ct * 128 : (ct + 1) * 128, kt * 128 : (kt + 1) * 128, 0, 0
            ].T

    # w_mrg[k, ct*6+t, m]: window (dy, dx) contracts the aligned pool-sum
    # (partitions 0:64, tap (dy, dx)) and the col-shifted copy (partitions
    # 64:128, tap (dy, dx+1); zero when dx+1 == 3).
    w_mrg = np.zeros((128, 12, 128), np.float32)
    for ct in range(2):
        for t, (dy, dx) in enumerate(MRG_WINS):
            w_mrg[0:64, ct * 6 + t, :] = w_add_s[
                ct * 128 : (ct + 1) * 128, :, dy, dx].T
            if dx + 1 < 3:
                w_mrg[64:128, ct * 6 + t, :] = w_add_s[
                    ct * 128 : (ct + 1) * 128, :, dy, dx + 1].T

    # w_post_l[k, co*18+kt*9+t, m] = w_post_s[co*128+m, kt*128+k, dy, dx]
    w_post_l = np.zeros((128, 36, 128), np.float32)
    for co in range(2):
        for kt in range(2):
            for t, (dy, dx) in enumerate(TAPS):
                w_post_l[:, co * 18 + kt * 9 + t, :] = w_post_s[
                    co * 128 : (co + 1) * 128, kt * 128 : (kt + 1) * 128, dy, dx
                ].T

    biases = np.zeros((128, 5), np.float32)
    biases[0:64, 0] = t_vpre
    biases[64:128, 0] = t_hpre
    t_mrg = t_res + t_add
    biases[:, 1] = t_mrg[0:128]
    biases[:, 2] = t_mrg[128:256]
    biases[:, 3] = t_post[0:128]
    biases[:, 4] = t_post[128:256]

    shared = {
        "zeros": np.zeros((128, 4 * WP), np.float32),
        "w_vh": w_vh,
        "w_res_l": w_res_l,
        "w_mrg": w_mrg,
        "w_post_l": w_post_l,
        "biases": biases,
    }
    return x, shared


def kernel(x, w_res, bn_res, w_vpre, bn_vpre, w_hpre, bn_hpre,
           w_add, bn_add, w_post, bn_post):
    from concourse.bass_utils import run_bass_kernel_spmd

    x, shared = _prep_inputs(x, w_res, bn_res, w_vpre, bn_vpre, w_hpre,
                             bn_hpre, w_add, bn_add, w_post, bn_post)

    if "nc" not in _CACHE:
        _CACHE["nc"] = build_nc()
    nc = _CACHE["nc"]

    in_maps = [dict(shared, x_s=np.ascontiguousarray(x[i])) for i in range(N_CORES)]
    res = run_bass_kernel_spmd(nc, in_maps, list(range(N_CORES)))
    return np.stack([res.results[i]["y"] for i in range(N_CORES)]).astype(np.float32)



# revision 18
# speedup vs baseline: 1.1894x; 1.0092x over previous
"""CornerPool block (conv/BN/cummax-pool residual block) on 8 Trainium2
NeuronCores, pure data-parallel over batch (1 sample per core).

Reference computation per sample (x: [256, 128, 128] f32):
    res    = BN(conv1x1(x, w_res))
    p1     = relu(BN(conv3x3(x, w_vpre)))        # 256 -> 64
    pool1  = reverse-cummax(p1, axis=H)          # TopPool
    p2     = relu(BN(conv3x3(x, w_hpre)))        # 256 -> 64
    pool2  = reverse-cummax(p2, axis=W)          # LeftPool
    merged = BN(conv3x3(pool1 + pool2, w_add))   # 64 -> 256
    out    = relu(res + merged)
    y      = relu(BN(conv3x3(out, w_post)))      # 256 -> 256

Kernel strategy (per core):
  * BN folded into conv weights/biases host-side; every conv is a
    sum-of-9-shifted-taps matmul accumulation in PSUM (channels on the
    partition dim, pixels on the free dim, N=512 = 4 image rows).
  * vpre+hpre convs fused into one matmul stream (same rhs windows,
    64+64 output channels fill the 128-wide stationary operand).
  * Pooling as in-place DVE tensor_max scans on the padded [128,130,130]
    conv-output buffer (p1 on partitions 0:64, p2 on 64:128).
  * The merged conv contracts over all 128 partitions with the 64-row
    weight block replicated, which computes conv(pool1 + pool2) without
    materializing the sum.
  * res 1x1 conv re-reads the phase-A x strips still live in the SBUF
    ring; accumulates into the same PSUM group as the merged conv.
  * out is bounced through DRAM in 4-row strips; the post conv streams
    it back with halo. All phases are emitted interleaved in reverse
    strip order so the Tile scheduler overlaps them into one wavefront.
  * All matmuls use float32r (full fp32 data, 1 cycle/row at N=512).
"""

import sys

import numpy as np

if "/opt/trn_rl_repo" not in sys.path:
    sys.path.insert(0, "/opt/trn_rl_repo")

EPS = 1e-5
C, M = 256, 64
B, H, W = 8, 128, 128
S = 4                      # output rows per strip
NS = H // S                # 32 strips
HP, WP = H + 2, W + 2      # padded spatial dims
N_CORES = 8

_CACHE = {}


def _patch_tile_drain():
    """This walrus build rejects >2 packed sync waits on the TileContext
    exit Drain. Split them into standalone wait_ge instructions."""
    import concourse.tile as tile
    from concourse.vector_clock import ScopedClock

    if getattr(tile.TileContext._drain_and_barrier, "_split_waits", False):
        return

    def _drain_and_barrier(self, tick_clock, wait_clock):
        nc = self.nc
        probe = nc.sync.nop(nofuse=True)
        wait_clock.add_sem_waits(
            probe.ins, ScopedClock({None: tick_clock.global_clock})
        )
        waits = list(probe.ins.sync_info.on_wait)
        if len(waits) > 1:
            probe.ins.sync_info.on_wait = waits[:1]
            sems_by_id = {s.num: s for s in wait_clock.sems.allocated().values()}
            for w in waits[1:]:
                nc.sync.wait_ge(sems_by_id[w.id], w.wait_value)
        nc.sync.drain()
        nc.all_engine_barrier()
        popped = nc._tile_sem_poison_stack.pop()
        assert popped is self._sem_poison
        nc.clear_and_free_semaphores(list(self.sems.allocated().values()))
        nc.all_engine_barrier()

    _drain_and_barrier._split_waits = True
    tile.TileContext._drain_and_barrier = _drain_and_barrier


TAPS = [(dy, dx) for dy in range(3) for dx in range(3)]
# merge-conv windows over the pool-sum buffer: (dy, 0) windows carry taps
# (dy,0) on the aligned half and (dy,1) on the col-shifted half; (dy, 2)
# windows carry tap (dy,2) with a zeroed shifted half.
MRG_WINS = [(0, 0), (1, 0), (2, 0), (0, 2), (1, 2), (2, 2)]


def _legalize_waits(nc, mybir):
    """This walrus build accepts at most ONE sync wait per instruction
    (any class). Split excess waits into single-wait NoOps emitted just
    before the instruction on the same engine sequencer."""
    for f in nc.m.functions:
        for bb in f.blocks:
            insts = bb.instructions
            out = []
            for inst in insts:
                si = inst.sync_info
                waits = list(si.on_wait) if si is not None else []
                if len(waits) > 1:
                    for j, w in enumerate(waits[:-1]):
                        noop = mybir.InstNoOp(
                            name=f"{inst.name}-ws{j}",
                            sync_info=mybir.SyncInfo(on_wait=[w], on_update=[]),
                            bass_nofuse=True,
                            engine=inst.engine,
                        )
                        nc.register_instruction(noop)
                        out.append(noop)
                    si.on_wait = waits[-1:]
                out.append(inst)
            insts[:] = out


def build_nc(debug_taps=False):
    import concourse.bass as bass
    import concourse.mybir as mybir
    import concourse.tile as tile

    _patch_tile_drain()
    f32 = mybir.dt.float32
    f32r = mybir.dt.float32r
    Relu = mybir.ActivationFunctionType.Relu

    nc = bass.Bass()
    x_d = nc.declare_dram_parameter("x_s", [C, H, WP], f32r, isOutput=False)
    # lhsT weight banks, laid out [k(part), idx, m]
    wvh_d = nc.declare_dram_parameter("w_vh", [128, 18, 128], f32r, isOutput=False)
    wres_d = nc.declare_dram_parameter("w_res_l", [128, 4, 128], f32r, isOutput=False)
    wmrg_d = nc.declare_dram_parameter("w_mrg", [128, 12, 128], f32r, isOutput=False)
    wpost_d = nc.declare_dram_parameter("w_post_l", [128, 36, 128], f32r, isOutput=False)
    bias_d = nc.declare_dram_parameter("biases", [128, 5], f32, isOutput=False)
    zeros_d = nc.declare_dram_parameter("zeros", [128, 4 * WP], f32r, isOutput=False)
    y_d = nc.declare_dram_parameter("y", [C, H, W], f32, isOutput=True)
    if debug_taps:
        dbg_pooled_d = nc.declare_dram_parameter(
            "dbg_pooled", [128, HP, WP], f32, isOutput=True)

    AluMax = mybir.AluOpType.max
    AluBypass = mybir.AluOpType.bypass
    NDB = 5                    # out-strip ring slots per channel-tile

    with tile.TileContext(nc) as tc:
        with (
            tc.tile_pool(name="const", bufs=1) as constp,
            tc.tile_pool(name="big", bufs=1) as bigp,
            tc.tile_pool(name="stage", bufs=6) as stagep,
            tc.tile_pool(name="psum", bufs=8, space="PSUM") as psump,
        ):
            # ---- constants on the phase-A critical path ----
            # Constants travel on the gpsimd SWDGE queues so they never
            # contend with the strip traffic on the 16 HWDGE queues.
            wvh = constp.tile([128, 18, 128], f32r)
            for j in range(0, 18, 3):
                nc.gpsimd.dma_start(wvh[:, j : j + 3, :], wvh_d[:, j : j + 3, :])
            bias = constp.tile([128, 5], f32)
            nc.gpsimd.dma_start(bias[:], bias_d[:])
            # SBUF zero strip: source for all pad-region fills (DVE copies;
            # memset is not ISA-legal for f32r, and strided zero-DMAs from
            # DRAM are catastrophically slow). On the sync queue so it lands
            # before the const weight banks clog gpsimd — the DVE pad-fill
            # queue head waits on it.
            zbuf = constp.tile([128, WP], f32r)
            nc.sync.dma_start(zbuf[:], zeros_d[:, :WP])
            # Dummy activation: triggers the one-time ACT_TABLE_LOAD
            # (~1.3us) during the DMA warmup instead of on the first real
            # strip's critical path.
            act_warm = constp.tile([128, 1], f32)
            nc.scalar.activation(act_warm[:], zbuf[:, 0:1], Relu)
            wres = constp.tile([128, 4, 128], f32r)
            wmrg = constp.tile([128, 12, 128], f32r)
            wpost = constp.tile([128, 36, 128], f32r)

            # ---- persistent buffers ----
            # conv-A output, padded; p1 on partitions 0:64, p2 on 64:128
            pooled = bigp.tile([128, HP, WP], f32r)

            def emit_deferred_consts():
                # conv pad rows/cols of `pooled`: rows via cheap contiguous
                # DMAs, the two pad columns via DVE copies from zbuf.
                nc.gpsimd.dma_start(pooled[:, 0, :], zeros_d[:, :WP])
                nc.gpsimd.dma_start(pooled[:, HP - 1, :], zeros_d[:, :WP])
                nc.vector.tensor_max(
                    pooled[:, 1 : HP - 1, 0], zbuf[:, : HP - 2], zbuf[:, : HP - 2])
                nc.vector.tensor_max(
                    pooled[:, 1 : HP - 1, WP - 1], zbuf[:, : HP - 2], zbuf[:, : HP - 2])
                nc.gpsimd.dma_start(wres[:], wres_d[:])
                for j in range(0, 12, 4):
                    e = min(j + 4, 12)
                    nc.gpsimd.dma_start(wmrg[:, j:e, :], wmrg_d[:, j:e, :])
                for j in range(0, 36, 5):
                    e = min(j + 5, 36)
                    nc.gpsimd.dma_start(wpost[:, j:e, :], wpost_d[:, j:e, :])

            # x strip ring for phase A: 4 slots x 2 channel-tiles
            xbuf = [
                [bigp.tile([128, S + 2, WP], f32r, name=f"xbuf{j}_{kt}")
                 for kt in range(2)]
                for j in range(4)
            ]
            # x strip ring for the res conv in phase C (full padded width
            # so the DMA stays contiguous; the matmul reads cols 1..128)
            cbuf = [
                [bigp.tile([128, S, WP], f32r, name=f"cbuf{j}_{kt}")
                 for kt in range(2)]
                for j in range(3)
            ]
            # staging ring for the pool-sum: p2 rows bounced through a
            # partition-remap DMA so the DVE can add them into the p1 half
            sbuf_tmp = [bigp.tile([128, S, WP], f32r, name=f"sum{j}")
                        for j in range(3)]
            # out strip ring between the merge conv and the post conv:
            # SBUF-resident, haloed. Tile j%NDB for strip j holds out rows
            # 4j-1 .. 4j+4 (indices 0..5); written directly by the C-phase
            # activations (no DRAM bounce), read by the D-phase matmuls.
            dbuf = [
                [bigp.tile([128, S + 2, WP], f32r, name=f"dbuf{j}_{ct}")
                 for ct in range(2)]
                for j in range(NDB)
            ]
            # dbuf pad columns 0 / WP-1 are read by the D-phase taps and
            # never written by the activations: zero them once.
            for j in range(NDB):
                for ct in range(2):
                    nc.vector.tensor_max(
                        dbuf[j][ct][:, :, 0], zbuf[:, : S + 2], zbuf[:, : S + 2])
                    nc.vector.tensor_max(
                        dbuf[j][ct][:, :, WP - 1], zbuf[:, : S + 2], zbuf[:, : S + 2])

            def emit_A(s):
                """conv(x, [w_vpre|w_hpre]) + BN + relu for rows 4s..4s+3,
                then the per-row LeftPool scans on the p2 half."""
                r = S * s
                xb = xbuf[s % 4]
                lo = max(0, r - 1)
                hi = min(H, r + S + 1)
                dst_lo = lo - (r - 1)
                for kt in range(2):
                    if s == 0:
                        # slot previously held a later strip's rows; row -1 pad
                        nc.sync.dma_start(xb[kt][:, 0, :], zeros_d[:, :WP])
                    elif s == NS - 1:
                        # first use of the slot: bottom halo row is pad
                        nc.sync.dma_start(xb[kt][:, S + 1, :], zeros_d[:, :WP])
                    nc.sync.dma_start(
                        xb[kt][:, dst_lo : dst_lo + (hi - lo), :],
                        x_d[kt * 128 : (kt + 1) * 128, lo:hi, :],
                    )
                ps = psump.tile([128, S * W], f32, tag="ps")
                n = len(TAPS) * 2
                i = 0
                for kt in range(2):
                    for t, (dy, dx) in enumerate(TAPS):
                        nc.tensor.matmul(
                            ps[:],
                            wvh[:, kt * 9 + t, :],
                            xb[kt][:, dy : dy + S, dx : dx + W],
                            start=(i == 0),
                            stop=(i == n - 1),
                        )
                        i += 1
                nc.scalar.activation(
                    pooled[:, r + 1 : r + 1 + S, 1 : 1 + W],
                    ps[:],
                    Relu,
                    bias=bias[:, 0:1],
                )
                # LeftPool (reverse cummax over W) for this strip's rows as
                # independent per-row hardware scans: no serial chain, and
                # p2 rows are final as soon as their strip lands.
                for ry in range(r + 1, r + 1 + S):
                    ap = pooled[64:128, ry, W : 0 : -1]
                    nc.vector.tensor_tensor_scan(
                        ap, ap, ap, 0.0, AluMax, AluBypass)

            def emit_toppool(s):
                r = S * s
                for y in range(min(H - 2, r + S - 1), r - 1, -1):
                    nc.vector.tensor_max(
                        pooled[0:64, y + 1, 1 : 1 + W],
                        pooled[0:64, y + 1, 1 : 1 + W],
                        pooled[0:64, y + 2, 1 : 1 + W],
                    )

            def emit_sum(m):
                """Collapse pooled rows 4m+1..4m+4 into the explicit pool
                sum: p1 half <- p1 + p2 (via a partition-remap bounce), then
                p2 half <- sum shifted one column left. The merge conv then
                contracts [sum(tap dx) ; sum(tap dx+1)] in one matmul, so 9
                taps need only 6 windows per output-channel tile."""
                rlo = S * m + 1
                t = sbuf_tmp[m % 3]
                nc.gpsimd.dma_start(t[0:64, :, :], pooled[64:128, rlo : rlo + S, :])
                nc.vector.tensor_add(
                    pooled[0:64, rlo : rlo + S, :],
                    pooled[0:64, rlo : rlo + S, :],
                    t[0:64, :, :],
                )
                nc.gpsimd.dma_start(
                    pooled[64:128, rlo : rlo + S, 0 : WP - 1],
                    pooled[0:64, rlo : rlo + S, 1:WP],
                )

            def emit_C(s):
                """res conv + merged conv + add + relu -> out ring tiles."""
                r = S * s
                cb = cbuf[s % 3]
                for kt in range(2):
                    nc.gpsimd.dma_start(
                        cb[kt][:],
                        x_d[kt * 128 : (kt + 1) * 128, r : r + S, :],
                    )
                for ct in range(2):
                    ps = psump.tile([128, S * W], f32, tag="ps")
                    for kt in range(2):
                        nc.tensor.matmul(
                            ps[:],
                            wres[:, ct * 2 + kt, :],
                            cb[kt][:, :, 1 : 1 + W],
                            start=(kt == 0),
                            stop=False,
                        )
                    for t, (dy, dx) in enumerate(MRG_WINS):
                        nc.tensor.matmul(
                            ps[:],
                            wmrg[:, ct * 6 + t, :],
                            pooled[:, r + dy : r + dy + S, dx : dx + W],
                            start=False,
                            stop=(t == 5),
                        )
                    b = bias[:, 1 + ct : 2 + ct]
                    # out rows 4s..4s+3 land in ring tile s (indices 1..4);
                    # row 4s also serves as tile s-1's bottom halo (index 5)
                    # and row 4s+3 as tile s+1's top halo (index 0).
                    nc.scalar.activation(
                        dbuf[s % NDB][ct][:, 1 : 1 + S, 1 : 1 + W],
                        ps[:], Relu, bias=b)
                    if s > 0:
                        nc.scalar.activation(
                            dbuf[(s - 1) % NDB][ct][:, S + 1, 1 : 1 + W],
                            ps[:, 0:W], Relu, bias=b)
                    if s < NS - 1:
                        nc.scalar.activation(
                            dbuf[(s + 1) % NDB][ct][:, 0, 1 : 1 + W],
                            ps[:, (S - 1) * W : S * W], Relu, bias=b)

            def emit_D(s):
                """post conv + BN + relu -> y strip."""
                r = S * s
                db = dbuf[s % NDB]
                for co in range(2):
                    ps = psump.tile([128, S * W], f32, tag="ps")
                    i = 0
                    for kt in range(2):
                        for t, (dy, dx) in enumerate(TAPS):
                            nc.tensor.matmul(
                                ps[:],
                                wpost[:, co * 18 + kt * 9 + t, :],
                                db[kt][:, dy : dy + S, dx : dx + W],
                                start=(i == 0),
                                stop=(i == 17),
                            )
                            i += 1
                    st = stagep.tile([128, S * W], f32, tag="std")
                    nc.scalar.activation(st[:], ps[:], Relu, bias=bias[:, 3 + co : 4 + co])
                    nc.scalar.dma_start(y_d[co * 128 : (co + 1) * 128, r : r + S, :], st[:])

            # Software-pipelined wavefront in groups of 8 strips, processed
            # bottom-up so the TopPool chain unlocks consumers as early as
            # possible. The C/D pairs for group k are emitted AFTER group
            # k-1's conv-A strips so the PE always has conv-A matmuls to
            # chew on while pools/activations for the C batch settle.
            def emit_group_A(k):
                for s in range(8 * k + 7, 8 * k - 1, -1):
                    emit_A(s)
                    emit_toppool(s)
                    # sum chunk s+1 is ready once toppool(s) has consumed
                    # the last original p1 row it overwrites
                    if s + 1 <= NS - 1:
                        emit_sum(s + 1)
                if k == 0:
                    emit_sum(0)

            def emit_group_CD(k):
                top = min(NS - 1, 8 * k + 8)
                for s in range(top, 8 * k, -1):
                    emit_C(s)
                    if s + 1 <= NS - 1:
                        emit_D(s + 1)

            emit_deferred_consts()
            # pad rows of the out ring: tile NS-1 index 5 is out row H
            # (zero), tile 0 index 0 is out row -1 (zero).
            for ct in range(2):
                nc.vector.tensor_max(
                    dbuf[(NS - 1) % NDB][ct][:, S + 1, :], zbuf[:], zbuf[:])
            emit_group_A(3)
            for k in range(3, -1, -1):
                if k > 0:
                    emit_group_A(k - 1)
                emit_group_CD(k)
            emit_C(0)
            emit_D(1)
            for ct in range(2):
                nc.vector.tensor_max(dbuf[0][ct][:, 0, :], zbuf[:], zbuf[:])
            emit_D(0)
            if debug_taps:
                nc.sync.dma_start(dbg_pooled_d[:], pooled[:])

    _legalize_waits(nc, mybir)
    return nc


def _fold_bn(w, bn):
    """BN(conv(x, w)) == conv(x, w * s[co]) + t[co]."""
    g, b, m, v = bn[0], bn[1], bn[2], bn[3]
    s = g / np.sqrt(v + EPS)
    t = b - m * s
    return w * s[:, None, None, None], t


def _prep_inputs(x, w_res, bn_res, w_vpre, bn_vpre, w_hpre, bn_hpre,
                 w_add, bn_add, w_post, bn_post):
    x = np.asarray(x, np.float32)
    xp = np.zeros((B, C, H, WP), np.float32)
    xp[:, :, :, 1 : 1 + W] = x
    x = xp
    w_res_s, t_res = _fold_bn(np.asarray(w_res, np.float32), np.asarray(bn_res, np.float32))
    w_vpre_s, t_vpre = _fold_bn(np.asarray(w_vpre, np.float32), np.asarray(bn_vpre, np.float32))
    w_hpre_s, t_hpre = _fold_bn(np.asarray(w_hpre, np.float32), np.asarray(bn_hpre, np.float32))
    w_add_s, t_add = _fold_bn(np.asarray(w_add, np.float32), np.asarray(bn_add, np.float32))
    w_post_s, t_post = _fold_bn(np.asarray(w_post, np.float32), np.asarray(bn_post, np.float32))

    # w_vh[k, kt*9+t, m]: m<64 vpre, m>=64 hpre; lhsT[k, m] = w[m, kt*128+k, dy, dx]
    w_vh = np.zeros((128, 18, 128), np.float32)
    for kt in range(2):
        for t, (dy, dx) in enumerate(TAPS):
            blk = kt * 128
            w_vh[:, kt * 9 + t, 0:64] = w_vpre_s[:, blk : blk + 128, dy, dx].T
            w_vh[:, kt * 9 + t, 64:128] = w_hpre_s[:, blk : blk + 128, dy, dx].T

    # w_res_l[k, ct*2+kt, m] = w_res_s[ct*128+m, kt*128+k]
    w_res_l = np.zeros((128, 4, 128), np.float32)
    for ct in range(2):
        for kt in range(2):
            w_res_l[:, ct * 2 + kt, :] = w_res_s[
                ct * 128 : (ct + 1) * 128, kt * 128 : (kt + 1) * 128, 0, 0
            ].T

    # w_mrg[k, ct*6+t, m]: window (dy, dx) contracts the aligned pool-sum
    # (partitions 0:64, tap (dy, dx)) and the col-shifted copy (partitions
    # 64:128, tap (dy, dx+1); zero when dx+1 == 3).
    w_mrg = np.zeros((128, 12, 128), np.float32)
    for ct in range(2):
        for t, (dy, dx) in enumerate(MRG_WINS):
            w_mrg[0:64, ct * 6 + t, :] = w_add_s[
                ct * 128 : (ct + 1) * 128, :, dy, dx].T
            if dx + 1 < 3:
                w_mrg[64:128, ct * 6 + t, :] = w_add_s[
                    ct * 128 : (ct + 1) * 128, :, dy, dx + 1].T

    # w_post_l[k, co*18+kt*9+t, m] = w_post_s[co*128+m, kt*128+k, dy, dx]
    w_post_l = np.zeros((128, 36, 128), np.float32)
    for co in range(2):
        for kt in range(2):
            for t, (dy, dx) in enumerate(TAPS):
                w_post_l[:, co * 18 + kt * 9 + t, :] = w_post_s[
                    co * 128 : (co + 1) * 128, kt * 128 : (kt + 1) * 128, dy, dx
                ].T

    biases = np.zeros((128, 5), np.float32)
    biases[0:64, 0] = t_vpre
    biases[64:128, 0] = t_hpre
    t_mrg = t_res + t_add
    biases[:, 1] = t_mrg[0:128]
    biases[:, 2] = t_mrg[128:256]
    biases[:, 3] = t_post[0:128]
    biases[:, 4] = t_post[128:256]

    shared = {
        "zeros": np.zeros((128, 4 * WP), np.float32),
        "w_vh": w_vh,
        "w_res_l": w_res_l,
        "w_mrg": w_mrg,
        "w_post_l": w_post_l,
        "biases": biases,
    }
    return x, shared


def kernel(x, w_res, bn_res, w_vpre, bn_vpre, w_hpre, bn_hpre,
           w_add, bn_add, w_post, bn_post):
    from concourse.bass_utils import run_bass_kernel_spmd

    x, shared = _prep_inputs(x, w_res, bn_res, w_vpre, bn_vpre, w_hpre,
                             bn_hpre, w_add, bn_add, w_post, bn_post)

    if "nc" not in _CACHE:
        _CACHE["nc"] = build_nc()
    nc = _CACHE["nc"]

    in_maps = [dict(shared, x_s=np.ascontiguousarray(x[i])) for i in range(N_CORES)]
    res = run_bass_kernel_spmd(nc, in_maps, list(range(N_CORES)))
    return np.stack([res.results[i]["y"] for i in range(N_CORES)]).astype(np.float32)



# revision 23
# speedup vs baseline: 1.1910x; 1.0014x over previous
"""CornerPool block (conv/BN/cummax-pool residual block) on 8 Trainium2
NeuronCores, pure data-parallel over batch (1 sample per core).

Reference computation per sample (x: [256, 128, 128] f32):
    res    = BN(conv1x1(x, w_res))
    p1     = relu(BN(conv3x3(x, w_vpre)))        # 256 -> 64
    pool1  = reverse-cummax(p1, axis=H)          # TopPool
    p2     = relu(BN(conv3x3(x, w_hpre)))        # 256 -> 64
    pool2  = reverse-cummax(p2, axis=W)          # LeftPool
    merged = BN(conv3x3(pool1 + pool2, w_add))   # 64 -> 256
    out    = relu(res + merged)
    y      = relu(BN(conv3x3(out, w_post)))      # 256 -> 256

Kernel strategy (per core):
  * BN folded into conv weights/biases host-side; every conv is a
    sum-of-9-shifted-taps matmul accumulation in PSUM (channels on the
    partition dim, pixels on the free dim, N=512 = 4 image rows).
  * vpre+hpre convs fused into one matmul stream (same rhs windows,
    64+64 output channels fill the 128-wide stationary operand).
  * Pooling as in-place DVE tensor_max scans on the padded [128,130,130]
    conv-output buffer (p1 on partitions 0:64, p2 on 64:128).
  * The merged conv contracts over all 128 partitions with the 64-row
    weight block replicated, which computes conv(pool1 + pool2) without
    materializing the sum.
  * res 1x1 conv re-reads the phase-A x strips still live in the SBUF
    ring; accumulates into the same PSUM group as the merged conv.
  * out is bounced through DRAM in 4-row strips; the post conv streams
    it back with halo. All phases are emitted interleaved in reverse
    strip order so the Tile scheduler overlaps them into one wavefront.
  * All matmuls use float32r (full fp32 data, 1 cycle/row at N=512).
"""

import sys

import numpy as np

if "/opt/trn_rl_repo" not in sys.path:
    sys.path.insert(0, "/opt/trn_rl_repo")

EPS = 1e-5
C, M = 256, 64
B, H, W = 8, 128, 128
S = 4                      # output rows per strip
NS = H // S                # 32 strips
HP, WP = H + 2, W + 2      # padded spatial dims
N_CORES = 8

_CACHE = {}


def _patch_tile_drain():
    """This walrus build rejects >2 packed sync waits on the TileContext
    exit Drain. Split them into standalone wait_ge instructions."""
    import concourse.tile as tile
    from concourse.vector_clock import ScopedClock

    if getattr(tile.TileContext._drain_and_barrier, "_split_waits", False):
        return

    def _drain_and_barrier(self, tick_clock, wait_clock):
        nc = self.nc
        probe = nc.sync.nop(nofuse=True)
        wait_clock.add_sem_waits(
            probe.ins, ScopedClock({None: tick_clock.global_clock})
        )
        waits = list(probe.ins.sync_info.on_wait)
        if len(waits) > 1:
            probe.ins.sync_info.on_wait = waits[:1]
            sems_by_id = {s.num: s for s in wait_clock.sems.allocated().values()}
            for w in waits[1:]:
                nc.sync.wait_ge(sems_by_id[w.id], w.wait_value)
        nc.sync.drain()
        nc.all_engine_barrier()
        popped = nc._tile_sem_poison_stack.pop()
        assert popped is self._sem_poison
        nc.clear_and_free_semaphores(list(self.sems.allocated().values()))
        nc.all_engine_barrier()

    _drain_and_barrier._split_waits = True
    tile.TileContext._drain_and_barrier = _drain_and_barrier


TAPS = [(dy, dx) for dy in range(3) for dx in range(3)]
# merge-conv windows over the pool-sum buffer: (dy, 0) windows carry taps
# (dy,0) on the aligned half and (dy,1) on the col-shifted half; (dy, 2)
# windows carry tap (dy,2) with a zeroed shifted half.
MRG_WINS = [(0, 0), (1, 0), (2, 0), (0, 2), (1, 2), (2, 2)]


def _legalize_waits(nc, mybir):
    """This walrus build accepts at most ONE sync wait per instruction
    (any class). Split excess waits into single-wait NoOps emitted just
    before the instruction on the same engine sequencer."""
    for f in nc.m.functions:
        for bb in f.blocks:
            insts = bb.instructions
            out = []
            for inst in insts:
                si = inst.sync_info
                waits = list(si.on_wait) if si is not None else []
                if len(waits) > 1:
                    for j, w in enumerate(waits[:-1]):
                        noop = mybir.InstNoOp(
                            name=f"{inst.name}-ws{j}",
                            sync_info=mybir.SyncInfo(on_wait=[w], on_update=[]),
                            bass_nofuse=True,
                            engine=inst.engine,
                        )
                        nc.register_instruction(noop)
                        out.append(noop)
                    si.on_wait = waits[-1:]
                out.append(inst)
            insts[:] = out


def build_nc(debug_taps=False):
    import concourse.bass as bass
    import concourse.mybir as mybir
    import concourse.tile as tile

    _patch_tile_drain()
    f32 = mybir.dt.float32
    f32r = mybir.dt.float32r
    Relu = mybir.ActivationFunctionType.Relu

    nc = bass.Bass()
    x_d = nc.declare_dram_parameter("x_s", [C, H, WP], f32r, isOutput=False)
    # lhsT weight banks, laid out [k(part), idx, m]
    wvh_d = nc.declare_dram_parameter("w_vh", [128, 18, 128], f32r, isOutput=False)
    wres_d = nc.declare_dram_parameter("w_res_l", [128, 4, 128], f32r, isOutput=False)
    wmrg_d = nc.declare_dram_parameter("w_mrg", [128, 12, 128], f32r, isOutput=False)
    wpost_d = nc.declare_dram_parameter("w_post_l", [128, 36, 128], f32r, isOutput=False)
    bias_d = nc.declare_dram_parameter("biases", [128, 5], f32, isOutput=False)
    zeros_d = nc.declare_dram_parameter("zeros", [128, 4 * WP], f32r, isOutput=False)
    y_d = nc.declare_dram_parameter("y", [C, H, W], f32, isOutput=True)
    if debug_taps:
        dbg_pooled_d = nc.declare_dram_parameter(
            "dbg_pooled", [128, HP, WP], f32, isOutput=True)

    AluMax = mybir.AluOpType.max
    AluBypass = mybir.AluOpType.bypass
    NDB = 5                    # out-strip ring slots per channel-tile

    with tile.TileContext(nc) as tc:
        with (
            tc.tile_pool(name="const", bufs=1) as constp,
            tc.tile_pool(name="big", bufs=1) as bigp,
            tc.tile_pool(name="stage", bufs=6) as stagep,
            tc.tile_pool(name="psum", bufs=8, space="PSUM") as psump,
        ):
            # ---- constants on the phase-A critical path ----
            # Constants travel on the gpsimd SWDGE queues so they never
            # contend with the strip traffic on the 16 HWDGE queues.
            wvh = constp.tile([128, 18, 128], f32r)
            for j in range(0, 18, 3):
                nc.gpsimd.dma_start(wvh[:, j : j + 3, :], wvh_d[:, j : j + 3, :])
            bias = constp.tile([128, 5], f32)
            nc.gpsimd.dma_start(bias[:], bias_d[:])
            # SBUF zero strip: source for all pad-region fills (DVE copies;
            # memset is not ISA-legal for f32r, and strided zero-DMAs from
            # DRAM are catastrophically slow). On the sync queue so it lands
            # before the const weight banks clog gpsimd — the DVE pad-fill
            # queue head waits on it.
            zbuf = constp.tile([128, WP], f32r)
            nc.scalar.dma_start(zbuf[:], zeros_d[:, :WP])
            # Dummy activation: triggers the one-time ACT_TABLE_LOAD
            # (~1.3us) during the DMA warmup instead of on the first real
            # strip's critical path.
            act_warm = constp.tile([128, 1], f32)
            nc.scalar.activation(act_warm[:], zbuf[:, 0:1], Relu)
            wres = constp.tile([128, 4, 128], f32r)
            wmrg = constp.tile([128, 12, 128], f32r)
            wpost = constp.tile([128, 36, 128], f32r)

            # ---- persistent buffers ----
            # conv-A output, padded; p1 on partitions 0:64, p2 on 64:128
            pooled = bigp.tile([128, HP, WP], f32r)

            def emit_deferred_consts():
                # conv pad rows/cols of `pooled`: rows via cheap contiguous
                # DMAs, the two pad columns via DVE copies from zbuf.
                nc.gpsimd.dma_start(pooled[:, 0, :], zeros_d[:, :WP])
                nc.gpsimd.dma_start(pooled[:, HP - 1, :], zeros_d[:, :WP])
                nc.vector.tensor_max(
                    pooled[:, 1 : HP - 1, 0], zbuf[:, : HP - 2], zbuf[:, : HP - 2])
                nc.vector.tensor_max(
                    pooled[:, 1 : HP - 1, WP - 1], zbuf[:, : HP - 2], zbuf[:, : HP - 2])
                nc.gpsimd.dma_start(wres[:], wres_d[:])
                for j in range(0, 12, 4):
                    e = min(j + 4, 12)
                    nc.gpsimd.dma_start(wmrg[:, j:e, :], wmrg_d[:, j:e, :])
                for j in range(0, 36, 5):
                    e = min(j + 5, 36)
                    nc.gpsimd.dma_start(wpost[:, j:e, :], wpost_d[:, j:e, :])

            # x strip ring for phase A: 4 slots x 2 channel-tiles
            xbuf = [
                [bigp.tile([128, S + 2, WP], f32r, name=f"xbuf{j}_{kt}")
                 for kt in range(2)]
                for j in range(4)
            ]
            # x strip ring for the res conv in phase C (full padded width
            # so the DMA stays contiguous; the matmul reads cols 1..128)
            cbuf = [
                [bigp.tile([128, S, WP], f32r, name=f"cbuf{j}_{kt}")
                 for kt in range(2)]
                for j in range(3)
            ]
            # staging ring for the pool-sum: p2 rows bounced through a
            # partition-remap DMA so the DVE can add them into the p1 half
            sbuf_tmp = [bigp.tile([128, S, WP], f32r, name=f"sum{j}")
                        for j in range(3)]
            # out strip ring between the merge conv and the post conv:
            # SBUF-resident, haloed. Tile j%NDB for strip j holds out rows
            # 4j-1 .. 4j+4 (indices 0..5); written directly by the C-phase
            # activations (no DRAM bounce), read by the D-phase matmuls.
            dbuf = [
                [bigp.tile([128, S + 2, WP], f32r, name=f"dbuf{j}_{ct}")
                 for ct in range(2)]
                for j in range(NDB)
            ]
            # dbuf pad columns 0 / WP-1 are read by the D-phase taps and
            # never written by the activations: zero them once.
            for j in range(NDB):
                for ct in range(2):
                    nc.vector.tensor_max(
                        dbuf[j][ct][:, :, 0], zbuf[:, : S + 2], zbuf[:, : S + 2])
                    nc.vector.tensor_max(
                        dbuf[j][ct][:, :, WP - 1], zbuf[:, : S + 2], zbuf[:, : S + 2])

            def emit_A(s):
                """conv(x, [w_vpre|w_hpre]) + BN + relu for rows 4s..4s+3,
                then the per-row LeftPool scans on the p2 half."""
                r = S * s
                xb = xbuf[s % 4]
                lo = max(0, r - 1)
                hi = min(H, r + S + 1)
                dst_lo = lo - (r - 1)
                # strip 30 loads ride the idle scalar queue so the first
                # two strips' inputs land in parallel at kernel start
                dma = nc.scalar.dma_start if s == NS - 2 else nc.sync.dma_start
                for kt in range(2):
                    dma(
                        xb[kt][:, dst_lo : dst_lo + (hi - lo), :],
                        x_d[kt * 128 : (kt + 1) * 128, lo:hi, :],
                    )
                for kt in range(2):
                    if s == 0:
                        # slot previously held a later strip's rows; row -1 pad
                        nc.sync.dma_start(xb[kt][:, 0, :], zeros_d[:, :WP])
                    elif s == NS - 1:
                        # first use of the slot: bottom halo row is pad
                        nc.sync.dma_start(xb[kt][:, S + 1, :], zeros_d[:, :WP])
                ps = psump.tile([128, S * W], f32, tag="ps")
                n = len(TAPS) * 2
                i = 0
                for kt in range(2):
                    for t, (dy, dx) in enumerate(TAPS):
                        nc.tensor.matmul(
                            ps[:],
                            wvh[:, kt * 9 + t, :],
                            xb[kt][:, dy : dy + S, dx : dx + W],
                            start=(i == 0),
                            stop=(i == n - 1),
                        )
                        i += 1
                nc.scalar.activation(
                    pooled[:, r + 1 : r + 1 + S, 1 : 1 + W],
                    ps[:],
                    Relu,
                    bias=bias[:, 0:1],
                )
                # LeftPool (reverse cummax over W) for this strip's rows as
                # independent per-row hardware scans: no serial chain, and
                # p2 rows are final as soon as their strip lands.
                for ry in range(r + 1, r + 1 + S):
                    ap = pooled[64:128, ry, W : 0 : -1]
                    nc.vector.tensor_tensor_scan(
                        ap, ap, ap, 0.0, AluMax, AluBypass)

            def emit_toppool(s):
                r = S * s
                for y in range(min(H - 2, r + S - 1), r - 1, -1):
                    nc.vector.tensor_max(
                        pooled[0:64, y + 1, 1 : 1 + W],
                        pooled[0:64, y + 1, 1 : 1 + W],
                        pooled[0:64, y + 2, 1 : 1 + W],
                    )

            def emit_sum_fetch(m):
                """Stage the scanned p2 rows of chunk m through a
                partition-remap bounce (only needs the LeftPool scans, so
                it is emitted before the toppool links to overlap them)."""
                rlo = S * m + 1
                t = sbuf_tmp[m % 3]
                nc.gpsimd.dma_start(t[0:64, :, :], pooled[64:128, rlo : rlo + S, :])

            def emit_sum_rest(m):
                """Collapse pooled rows 4m+1..4m+4 into the explicit pool
                sum: p1 half <- p1 + p2, then p2 half <- sum shifted one
                column left. The merge conv then contracts [sum(tap dx) ;
                sum(tap dx+1)] in one matmul, so 9 taps need only 6 windows
                per output-channel tile."""
                rlo = S * m + 1
                t = sbuf_tmp[m % 3]
                nc.vector.tensor_add(
                    pooled[0:64, rlo : rlo + S, :],
                    pooled[0:64, rlo : rlo + S, :],
                    t[0:64, :, :],
                )
                nc.gpsimd.dma_start(
                    pooled[64:128, rlo : rlo + S, 0 : WP - 1],
                    pooled[0:64, rlo : rlo + S, 1:WP],
                )

            def emit_C(s):
                """res conv + merged conv + add + relu -> out ring tiles."""
                r = S * s
                cb = cbuf[s % 3]
                for kt in range(2):
                    nc.gpsimd.dma_start(
                        cb[kt][:],
                        x_d[kt * 128 : (kt + 1) * 128, r : r + S, :],
                    )
                for ct in range(2):
                    ps = psump.tile([128, S * W], f32, tag="ps")
                    for kt in range(2):
                        nc.tensor.matmul(
                            ps[:],
                            wres[:, ct * 2 + kt, :],
                            cb[kt][:, :, 1 : 1 + W],
                            start=(kt == 0),
                            stop=False,
                        )
                    for t, (dy, dx) in enumerate(MRG_WINS):
                        nc.tensor.matmul(
                            ps[:],
                            wmrg[:, ct * 6 + t, :],
                            pooled[:, r + dy : r + dy + S, dx : dx + W],
                            start=False,
                            stop=(t == 5),
                        )
                    b = bias[:, 1 + ct : 2 + ct]
                    # out rows 4s..4s+3 land in ring tile s (indices 1..4);
                    # row 4s also serves as tile s-1's bottom halo (index 5)
                    # and row 4s+3 as tile s+1's top halo (index 0).
                    nc.scalar.activation(
                        dbuf[s % NDB][ct][:, 1 : 1 + S, 1 : 1 + W],
                        ps[:], Relu, bias=b)
                    if s > 0:
                        nc.scalar.activation(
                            dbuf[(s - 1) % NDB][ct][:, S + 1, 1 : 1 + W],
                            ps[:, 0:W], Relu, bias=b)
                    if s < NS - 1:
                        nc.scalar.activation(
                            dbuf[(s + 1) % NDB][ct][:, 0, 1 : 1 + W],
                            ps[:, (S - 1) * W : S * W], Relu, bias=b)

            def emit_D(s):
                """post conv + BN + relu -> y strip."""
                r = S * s
                db = dbuf[s % NDB]
                for co in range(2):
                    ps = psump.tile([128, S * W], f32, tag="ps")
                    i = 0
                    for kt in range(2):
                        for t, (dy, dx) in enumerate(TAPS):
                            nc.tensor.matmul(
                                ps[:],
                                wpost[:, co * 18 + kt * 9 + t, :],
                                db[kt][:, dy : dy + S, dx : dx + W],
                                start=(i == 0),
                                stop=(i == 17),
                            )
                            i += 1
                    st = stagep.tile([128, S * W], f32, tag="std")
                    nc.scalar.activation(st[:], ps[:], Relu, bias=bias[:, 3 + co : 4 + co])
                    nc.scalar.dma_start(y_d[co * 128 : (co + 1) * 128, r : r + S, :], st[:])

            # Software-pipelined wavefront in groups of 8 strips, processed
            # bottom-up so the TopPool chain unlocks consumers as early as
            # possible. The C/D pairs for group k are emitted AFTER group
            # k-1's conv-A strips so the PE always has conv-A matmuls to
            # chew on while pools/activations for the C batch settle.
            def emit_group_A(k):
                for s in range(8 * k + 7, 8 * k - 1, -1):
                    emit_A(s)
                    if s + 1 <= NS - 1:
                        emit_sum_fetch(s + 1)
                    emit_toppool(s)
                    # the sum add overwrites p1 rows toppool(s) just read,
                    # so it trails the toppool links
                    if s + 1 <= NS - 1:
                        emit_sum_rest(s + 1)
                if k == 0:
                    emit_sum_fetch(0)
                    emit_sum_rest(0)

            def emit_group_CD(k):
                top = min(NS - 1, 8 * k + 8)
                for s in range(top, 8 * k, -1):
                    emit_C(s)
                    if s + 1 <= NS - 1:
                        emit_D(s + 1)

            emit_deferred_consts()
            # pad rows of the out ring: tile NS-1 index 5 is out row H
            # (zero), tile 0 index 0 is out row -1 (zero).
            for ct in range(2):
                nc.vector.tensor_max(
                    dbuf[(NS - 1) % NDB][ct][:, S + 1, :], zbuf[:], zbuf[:])
            emit_group_A(3)
            for k in range(3, -1, -1):
                if k > 0:
                    emit_group_A(k - 1)
                emit_group_CD(k)
            emit_C(0)
            emit_D(1)
            for ct in range(2):
                nc.vector.tensor_max(dbuf[0][ct][:, 0, :], zbuf[:], zbuf[:])
            emit_D(0)
            if debug_taps:
                nc.sync.dma_start(dbg_pooled_d[:], pooled[:])

    _legalize_waits(nc, mybir)
    return nc


def _fold_bn(w, bn):
    """BN(conv(x, w)) == conv(x, w * s[co]) + t[co]."""
    g, b, m, v = bn[0], bn[1], bn[2], bn[3]
    s = g / np.sqrt(v + EPS)
    t = b - m * s
    return w * s[:, None, None, None], t


def _prep_inputs(x, w_res, bn_res, w_vpre, bn_vpre, w_hpre, bn_hpre,
                 w_add, bn_add, w_post, bn_post):
    x = np.asarray(x, np.float32)
    xp = np.zeros((B, C, H, WP), np.float32)
    xp[:, :, :, 1 : 1 + W] = x
    x = xp
    w_res_s, t_res = _fold_bn(np.asarray(w_res, np.float32), np.asarray(bn_res, np.float32))
    w_vpre_s, t_vpre = _fold_bn(np.asarray(w_vpre, np.float32), np.asarray(bn_vpre, np.float32))
    w_hpre_s, t_hpre = _fold_bn(np.asarray(w_hpre, np.float32), np.asarray(bn_hpre, np.float32))
    w_add_s, t_add = _fold_bn(np.asarray(w_add, np.float32), np.asarray(bn_add, np.float32))
    w_post_s, t_post = _fold_bn(np.asarray(w_post, np.float32), np.asarray(bn_post, np.float32))

    # w_vh[k, kt*9+t, m]: m<64 vpre, m>=64 hpre; lhsT[k, m] = w[m, kt*128+k, dy, dx]
    w_vh = np.zeros((128, 18, 128), np.float32)
    for kt in range(2):
        for t, (dy, dx) in enumerate(TAPS):
            blk = kt * 128
            w_vh[:, kt * 9 + t, 0:64] = w_vpre_s[:, blk : blk + 128, dy, dx].T
            w_vh[:, kt * 9 + t, 64:128] = w_hpre_s[:, blk : blk + 128, dy, dx].T

    # w_res_l[k, ct*2+kt, m] = w_res_s[ct*128+m, kt*128+k]
    w_res_l = np.zeros((128, 4, 128), np.float32)
    for ct in range(2):
        for kt in range(2):
            w_res_l[:, ct * 2 + kt, :] = w_res_s[
                ct * 128 : (ct + 1) * 128, kt * 128 : (kt + 1) * 128, 0, 0
            ].T

    # w_mrg[k, ct*6+t, m]: window (dy, dx) contracts the aligned pool-sum
    # (partitions 0:64, tap (dy, dx)) and the col-shifted copy (partitions
    # 64:128, tap (dy, dx+1); zero when dx+1 == 3).
    w_mrg = np.zeros((128, 12, 128), np.float32)
    for ct in range(2):
        for t, (dy, dx) in enumerate(MRG_WINS):
            w_mrg[0:64, ct * 6 + t, :] = w_add_s[
                ct * 128 : (ct + 1) * 128, :, dy, dx].T
            if dx + 1 < 3:
                w_mrg[64:128, ct * 6 + t, :] = w_add_s[
                    ct * 128 : (ct + 1) * 128, :, dy, dx + 1].T

    # w_post_l[k, co*18+kt*9+t, m] = w_post_s[co*128+m, kt*128+k, dy, dx]
    w_post_l = np.zeros((128, 36, 128), np.float32)
    for co in range(2):
        for kt in range(2):
            for t, (dy, dx) in enumerate(TAPS):
                w_post_l[:, co * 18 + kt * 9 + t, :] = w_post_s[
                    co * 128 : (co + 1) * 128, kt * 128 : (kt + 1) * 128, dy, dx
                ].T

    biases = np.zeros((128, 5), np.float32)
    biases[0:64, 0] = t_vpre
    biases[64:128, 0] = t_hpre
    t_mrg = t_res + t_add
    biases[:, 1] = t_mrg[0:128]
    biases[:, 2] = t_mrg[128:256]
    biases[:, 3] = t_post[0:128]
    biases[:, 4] = t_post[128:256]

    shared = {
        "zeros": np.zeros((128, 4 * WP), np.float32),
        "w_vh": w_vh,
        "w_res_l": w_res_l,
        "w_mrg": w_mrg,
        "w_post_l": w_post_l,
        "biases": biases,
    }
    return x, shared


def kernel(x, w_res, bn_res, w_vpre, bn_vpre, w_hpre, bn_hpre,
           w_add, bn_add, w_post, bn_post):
    from concourse.bass_utils import run_bass_kernel_spmd

    x, shared = _prep_inputs(x, w_res, bn_res, w_vpre, bn_vpre, w_hpre,
                             bn_hpre, w_add, bn_add, w_post, bn_post)

    if "nc" not in _CACHE:
        _CACHE["nc"] = build_nc()
    nc = _CACHE["nc"]

    in_maps = [dict(shared, x_s=np.ascontiguousarray(x[i])) for i in range(N_CORES)]
    res = run_bass_kernel_spmd(nc, in_maps, list(range(N_CORES)))
    return np.stack([res.results[i]["y"] for i in range(N_CORES)]).astype(np.float32)

